# revision 7
# baseline (speedup 1.0000x reference)
"""Bass/Trainium2 kernel for nn_BaysianMLPMaskedDropout (8 NeuronCores).

Sharding: 4 data-parallel groups (batch 1024 -> 4 x 256) x 2-way tensor
parallel (the 4096 hidden axis of each BayesLinear split in halves).
Activations are kept feature-major ("transposed", [feature, batch]) on chip so
every matmul contracts along the partition axis with no transposes between
layers.  The per-pair ReduceScatter after layer 1 and AllReduce after the
output layer run on the collective engine.  The scalar flow-loss pieces are
reduced on-device into per-core partial vectors and combined on the host.

log(pw) = log(cdf(w+eps)-cdf(w-eps)) is evaluated with the exact expansion
log(2*EPS/sqrt(2pi)) - w_ls - eps^2/2 (error < 1e-7, far below the f32 noise
of the reference's catastrophically-cancelling cdf difference).
"""

import numpy as np

import concourse.bacc as bacc
import concourse.tile as tile
from concourse import mybir
from concourse.alu_op_type import AluOpType
from concourse.bass_utils import run_bass_kernel_spmd

F16 = mybir.dt.float16
F32 = mybir.dt.float32
AF = mybir.ActivationFunctionType
AX = mybir.AxisListType.X

B, IN, H0, H1, OUT = 1024, 2048, 4096, 4096, 1000
OUTP = 1024                      # classes padded to 8*128 for uniform tiles
TP, DP = 2, 4
BL = B // DP                     # 256 batch rows per DP group
H0H, H1H = H0 // TP, H1 // TP    # 2048 per rank
NK_IN = IN // 128                # 16 contraction chunks, layer 0
NK_H = H0H // 128                # 16 contraction chunks, layers 1/o
G0, G1, GO = H0H // 512, H1 // 512, OUTP // 512   # 4, 8, 2 M-groups
NBLK = (G0 + G1) * NK_IN         # weight-stream blocks with flow sums (192)
EPS_CDF = 1e-5
C0 = float(np.log(2 * EPS_CDF / np.sqrt(2 * np.pi)))
NEG = 0.01
DECAY = 0.01
CLIP_LO = np.float32(1e-6)
CLIP_HI = np.float32(1.0 - 1e-6)

_CACHE = {}


def _build():
    nc = bacc.Bacc("TRN2", debug=False, num_devices=8, enable_asserts=False)

    def din(name, shape, dt=F16):
        return nc.dram_tensor(name, shape, dt, kind="ExternalInput").ap()

    wp0 = din("wp0", [G0 * NK_IN * 128, 1536])
    wp1 = din("wp1", [G1 * NK_H * 128, 1536])
    wpo = din("wpo", [GO * NK_H * 128, 1536])
    xT = din("xT", [IN, BL])
    pf0 = din("pf0", [H0H, BL])
    pf1 = din("pf1", [H1H, BL])
    mz0 = din("mz0", [H0H, BL])
    mz1 = din("mz1", [H1H, BL])
    bv0 = din("bv0", [128, 48], F32)   # [mu | ls | eps] each [128,16]
    bv1 = din("bv1", [128, 48], F32)
    bvo = din("bvo", [128, 24], F32)   # padded to 1024 classes
    mk0 = din("mk0", [128, 16], F32)
    mk1 = din("mk1", [128, 16], F32)
    zw2 = din("zw2", [128, 16], F32)
    y2 = din("y2", [128, 2], F32)
    ar1 = din("ar1", [1, OUTP], F32)
    p128 = din("p128", [128, 1], F32)

    pred_out = nc.dram_tensor("pred_out", [BL, OUT], F32, kind="ExternalOutput").ap()
    partials = nc.dram_tensor("partials", [128, 16], F32, kind="ExternalOutput").ap()

    groups = [[0, 1], [2, 3], [4, 5], [6, 7]]

    with tile.TileContext(nc) as tc:
        with (
            tc.tile_pool(name="sb", bufs=1) as sb,
            tc.tile_pool(name="ps", bufs=1, space="PSUM") as ps,
            tc.tile_pool(name="dram", bufs=1, space="DRAM") as dram,
        ):
            # ---------------- small constant loads ----------------
            def load(name, src, shape, dt=F32, bufs=1):
                t = sb.tile(shape, dt, tag=name, bufs=bufs)
                nc.sync.dma_start(t[:], src)
                return t

            bv0_t = load("bv0", bv0[:, :], [128, 48])
            bv1_t = load("bv1", bv1[:, :], [128, 48])
            bvo_t = load("bvo", bvo[:, :], [128, 24])
            mk0_t = load("mk0", mk0[:, :], [128, 16])
            mk1_t = load("mk1", mk1[:, :], [128, 16])
            zw_t = load("zw", zw2[:, :], [128, 16])
            y_t = load("y", y2[:, :], [128, 2])
            p128_t = load("p128", p128[:, :], [128, 1])
            arbc = sb.tile([128, OUTP], F32, tag="arbc")
            nc.sync.dma_start(arbc[:], ar1[0:1, :].partition_broadcast(128))

            ones_t = sb.tile([128, 1], F16, tag="ones")
            nc.vector.memset(ones_t[:], 1.0)
            negones_t = sb.tile([128, 1], F16, tag="negones")
            nc.vector.memset(negones_t[:], -1.0)
            cent_t = sb.tile([128, 1], F16, tag="cent")
            nc.vector.memset(cent_t[:], DECAY)

            parts_t = sb.tile([128, 16], F32, tag="parts")
            nc.vector.memset(parts_t[:], 0.0)

            # bias materialization: b = b_mu + exp(b_ls) * b_eps
            def bias_mat(bv, ncol, tag):
                s = sb.tile([128, ncol], F32, tag=tag + "s")
                nc.scalar.activation(s[:], bv[:, ncol:2 * ncol], AF.Exp)
                t = sb.tile([128, ncol], F32, tag=tag + "t")
                nc.vector.tensor_tensor(t[:], s[:], bv[:, 2 * ncol:3 * ncol], AluOpType.mult)
                b = sb.tile([128, ncol], F32, tag=tag + "b")
                nc.vector.tensor_tensor(b[:], t[:], bv[:, 0:ncol], AluOpType.add)
                return b

            b0m = bias_mat(bv0_t, 16, "b0")
            b1m = bias_mat(bv1_t, 16, "b1")
            bom = bias_mat(bvo_t, 8, "bo")

            # b-vector flow partials: sum(b_ls), sum(b_eps^2) for layers 0,1
            nc.vector.tensor_reduce(parts_t[:, 2:3], bv0_t[:, 16:32], AX, AluOpType.add)
            nc.vector.tensor_reduce(parts_t[:, 3:4], bv1_t[:, 16:32], AX, AluOpType.add)
            jb = sb.tile([128, 16], F32, tag="jb", bufs=2)
            nc.vector.affine_mul_reduce(jb[:], parts_t[:, 4:5], bv0_t[:, 32:48],
                                        bv0_t[:, 32:48], 1.0, 0.0)
            jb2 = sb.tile([128, 16], F32, tag="jb", bufs=2)
            nc.vector.affine_mul_reduce(jb2[:], parts_t[:, 5:6], bv1_t[:, 32:48],
                                        bv1_t[:, 32:48], 1.0, 0.0)

            # ---------------- x load + xz partial ----------------
            xall = sb.tile([128, NK_IN * BL], F16, tag="xall")
            nc.sync.dma_start(
                xall[:].rearrange("p (k b) -> p k b", k=NK_IN),
                xT[:, :].rearrange("(k p) b -> p k b", p=128),
            )
            xzacc = sb.tile([128, 16], F32, tag="xzacc")
            jx = None
            for k in range(NK_IN):
                jx = sb.tile([128, BL], F16, tag="jx", bufs=2)
                nc.vector.tensor_scalar(jx[:], xall[:, k * BL:(k + 1) * BL],
                                        zw_t[:, k:k + 1], 0.0, AluOpType.mult,
                                        AluOpType.add,
                                        accum_out=xzacc[:, k:k + 1])
            nc.vector.tensor_reduce(parts_t[:, 7:8], xzacc[:, :], AX, AluOpType.add)

            # ---------------- masks + pfz flow (both layers) ----------------
            flacc = sb.tile([128, 64], F32, tag="flacc")
            ps_ms = ps.tile([1, 512], F32, tag="psms")

            def mask_flow(mzap, pfap, mk_t, layer, nk):
                mzall = sb.tile([128, nk * BL], F16, tag=f"mzall{layer}")
                nc.sync.dma_start(
                    mzall[:].rearrange("p (k b) -> p k b", k=nk),
                    mzap.rearrange("(k p) b -> p k b", p=128),
                )
                pfall = sb.tile([128, nk * BL], F16, tag=f"pfall{layer}")
                nc.sync.dma_start(
                    pfall[:].rearrange("p (k b) -> p k b", k=nk),
                    pfap.rearrange("(k p) b -> p k b", p=128),
                )
                m_tiles = []
                for k in range(nk):
                    mzs = mzall[:, k * BL:(k + 1) * BL]
                    pc = sb.tile([128, BL], F32, tag="pc", bufs=3)
                    nc.vector.tensor_scalar(pc[:], pfall[:, k * BL:(k + 1) * BL],
                                            float(CLIP_LO), float(CLIP_HI),
                                            AluOpType.max, AluOpType.min)
                    lnp = sb.tile([128, BL], F32, tag="lnp", bufs=3)
                    nc.scalar.activation(lnp[:], pc[:], AF.Ln)
                    ln1p = sb.tile([128, BL], F32, tag="ln1p", bufs=3)
                    nc.scalar.activation(ln1p[:], pc[:], AF.Ln, bias=1.0, scale=-1.0,
                                         accum_out=flacc[:, layer * 32 + k:layer * 32 + k + 1])
                    d = sb.tile([128, BL], F32, tag="dln", bufs=3)
                    nc.vector.tensor_tensor(d[:], lnp[:], ln1p[:], AluOpType.subtract)
                    jf = sb.tile([128, BL], F32, tag="jf", bufs=2)
                    nc.vector.affine_mul_reduce(
                        jf[:], flacc[:, layer * 32 + 16 + k:layer * 32 + 16 + k + 1],
                        mzs, d[:], 1.0, 0.0)
                    mt = sb.tile([128, BL], F16, tag=f"m{layer}_{k}")
                    nc.vector.tensor_scalar(mt[:], mzs, mk_t[:, k:k + 1], None,
                                            AluOpType.mult)
                    nc.tensor.matmul(ps_ms[:, layer * BL:(layer + 1) * BL], ones_t[:], mt[:],
                                     start=(k == 0), stop=(k == nk - 1))
                    m_tiles.append(mt)
                return m_tiles

            m0_tiles = mask_flow(mz0[:, :], pf0[:, :], mk0_t, 0, NK_H)
            m1_tiles = mask_flow(mz1[:, :], pf1[:, :], mk1_t, 1, NK_H)
            nc.vector.tensor_reduce(parts_t[:, 6:7], flacc[:, :], AX, AluOpType.add)

            # msum AllReduce (pair) -> mult factors broadcast to 128 partitions
            ms_sb = sb.tile([1, 512], F32, tag="ms_sb")
            nc.vector.tensor_copy(ms_sb[:], ps_ms[:])
            cc_ms_in = dram.tile([1, 512], F32)
            cc_ms_out = dram.tile([1, 512], F32)
            nc.gpsimd.dma_start(cc_ms_in[:], ms_sb[:])
            nc.gpsimd.collective_compute("AllReduce", AluOpType.add,
                                         replica_groups=groups,
                                         ins=[cc_ms_in.opt()], outs=[cc_ms_out.opt()])
            msbc = sb.tile([128, 512], F32, tag="msbc")
            nc.sync.dma_start(msbc[:], cc_ms_out[0:1, :].partition_broadcast(128))
            multbc = sb.tile([128, 512], F32, tag="multbc")
            nc.vector.tensor_scalar(multbc[:], msbc[:], 1e-6, None, AluOpType.add)
            nc.vector.reciprocal(multbc[:], multbc[:])
            nc.vector.tensor_scalar(multbc[:], multbc[:], float(H0), None, AluOpType.mult)
            mult0bc = multbc[:, 0:BL]
            mult1bc = multbc[:, BL:2 * BL]

            # ---------------- streaming weight layers ----------------
            expacc = sb.tile([128, NBLK], F32, tag="expacc")
            ps_w = ps.tile([1, 512], F32, tag="psw")

            def wstream(wp, G, nk, rhs_sl, with_sums, blk0, evict):
                for g in range(G):
                    pts = [ps.tile([128, BL], F32, tag="psmain", bufs=4,
                                   name=f"psm{g}_{m}") for m in range(4)]
                    for k in range(nk):
                        blk = sb.tile([128, 1536], F16, tag="wblk", bufs=4)
                        row = ((g * nk + k) * 128)
                        dma_eng = nc.sync if (k % 2 == 0) else nc.gpsimd
                        dma_eng.dma_start(blk[:], wp[row:row + 128, :])
                        mu, ls, eps = blk[:, 0:512], blk[:, 512:1024], blk[:, 1024:1536]
                        s_t = sb.tile([128, 512], F16, tag="s", bufs=3)
                        bi = blk0 + g * nk + k
                        nc.scalar.activation(
                            s_t[:], ls, AF.Exp,
                            accum_out=(expacc[:, bi:bi + 1] if with_sums else None))
                        t_t = sb.tile([128, 512], F16, tag="t", bufs=3)
                        nc.vector.tensor_tensor(t_t[:], s_t[:], eps, AluOpType.mult)
                        w_t = sb.tile([128, 512], F16, tag="w", bufs=3)
                        nc.vector.tensor_tensor(w_t[:], t_t[:], mu, AluOpType.add)
                        if with_sums:
                            first = bi == 0
                            # -sum(ls) via negated ones
                            nc.tensor.matmul(ps_w[:], negones_t[:], ls,
                                             start=first, stop=False,
                                             skip_group_check=True)
                            # 0.01*mu^2 (gpsimd) and -0.5*eps^2 (DVE)
                            mu2 = sb.tile([128, 512], F16, tag="mu2", bufs=2)
                            nc.gpsimd.tensor_tensor(mu2[:], mu, mu, AluOpType.mult)
                            eps2 = sb.tile([128, 512], F16, tag="eps2", bufs=2)
                            nc.vector.scalar_tensor_tensor(eps2[:], eps, -0.5, eps,
                                                           AluOpType.mult, AluOpType.mult)
                            nc.tensor.matmul(ps_w[:], cent_t[:], mu2[:],
                                             start=False, stop=False,
                                             skip_group_check=True)
                            last = bi == NBLK - 1
                            nc.tensor.matmul(ps_w[:], ones_t[:], eps2[:],
                                             start=False, stop=last,
                                             skip_group_check=True)
                        for m in range(4):
                            nc.tensor.matmul(pts[m][:], w_t[:, m * 128:(m + 1) * 128],
                                             rhs_sl(k), start=(k == 0), stop=(k == nk - 1))
                    for m in range(4):
                        evict(g * 4 + m, pts[m])
                return

            # ---- layer 0 ----
            h0m_tiles = [None] * NK_H

            def evict0(j, pt):
                lr = sb.tile([128, BL], F32, tag="h0lr", bufs=2)
                nc.scalar.activation(lr[:], pt[:], AF.Lrelu, bias=b0m[:, j:j + 1],
                                     alpha=NEG)
                hm = sb.tile([128, BL], F16, tag=f"h0m{j}")
                nc.vector.tensor_tensor(hm[:], lr[:], m0_tiles[j][:], AluOpType.mult)
                h0m_tiles[j] = hm

            wstream(wp0, G0, NK_IN, lambda k: xall[:, k * BL:(k + 1) * BL],
                    True, 0, evict0)

            # ---- layer 1 (partials over my H0 half, full H1) ----
            cc_h1in = dram.tile([H1, BL], F16)
            cc_h1rs = dram.tile([H1H, BL], F16)

            def evict1(j, pt):
                hp = sb.tile([128, BL], F16, tag="h1p", bufs=4)
                nc.vector.tensor_copy(hp[:], pt[:])
                nc.gpsimd.dma_start(cc_h1in[j * 128:(j + 1) * 128, :], hp[:])

            wstream(wp1, G1, NK_H, lambda k: h0m_tiles[k][:],
                    True, G0 * NK_IN, evict1)

            nc.gpsimd.collective_compute("ReduceScatter", AluOpType.add,
                                         replica_groups=groups,
                                         ins=[cc_h1in.opt()], outs=[cc_h1rs.opt()])

            h1s = sb.tile([128, NK_H * BL], F16, tag="h1s")
            nc.sync.dma_start(
                h1s[:].rearrange("p (k b) -> p k b", k=NK_H),
                cc_h1rs[:, :].rearrange("(k p) b -> p k b", p=128),
            )
            h1m_tiles = []
            for k in range(NK_H):
                a = sb.tile([128, BL], F32, tag="h1a", bufs=2)
                nc.vector.tensor_tensor(a[:], h1s[:, k * BL:(k + 1) * BL], mult0bc,
                                        AluOpType.mult)
                lr = sb.tile([128, BL], F32, tag="h1lr", bufs=2)
                nc.scalar.activation(lr[:], a[:], AF.Lrelu, bias=b1m[:, k:k + 1],
                                     alpha=NEG)
                c = sb.tile([128, BL], F32, tag="h1c", bufs=2)
                nc.vector.tensor_scalar(c[:], lr[:], mk1_t[:, k:k + 1], None,
                                        AluOpType.mult)
                hm = sb.tile([128, BL], F16, tag=f"h1m{k}")
                nc.vector.tensor_tensor(hm[:], c[:], m1_tiles[k][:], AluOpType.mult)
                h1m_tiles.append(hm)

            # ---- output layer ----
            cc_pin = dram.tile([OUTP, BL], F32)
            cc_pout = dram.tile([OUTP, BL], F32)

            def evicto(j, pt):
                pp = sb.tile([128, BL], F32, tag="pp", bufs=4)
                nc.scalar.activation(pp[:], pt[:], AF.Copy)
                nc.gpsimd.dma_start(cc_pin[j * 128:(j + 1) * 128, :], pp[:])

            wstream(wpo, GO, NK_H, lambda k: h1m_tiles[k][:],
                    False, 0, evicto)

            nc.gpsimd.collective_compute("AllReduce", AluOpType.add,
                                         replica_groups=groups,
                                         ins=[cc_pin.opt()], outs=[cc_pout.opt()])

            par = sb.tile([128, (OUTP // 128) * BL], F32, tag="par")
            nc.sync.dma_start(
                par[:].rearrange("p (k b) -> p k b", k=OUTP // 128),
                cc_pout[:, :].rearrange("(k p) b -> p k b", p=128),
            )
            predT_tiles = []
            for m in range(OUTP // 128):
                pm = sb.tile([128, BL], F32, tag="pTa", bufs=2)
                nc.vector.tensor_tensor(pm[:], par[:, m * BL:(m + 1) * BL], mult1bc,
                                        AluOpType.mult)
                pt2 = sb.tile([128, BL], F32, tag=f"pT{m}")
                nc.vector.tensor_scalar(pt2[:], pm[:], bom[:, m:m + 1], None,
                                        AluOpType.add)
                predT_tiles.append(pt2)

            # transpose predT [1024, 256] -> pred [256, 1024]
            idn = sb.tile([128, 128], F32, tag="idn")
            nc.vector.tensor_scalar(idn[:], arbc[:, 0:128], p128_t[:, 0:1], None,
                                    AluOpType.is_equal)
            predsb = [sb.tile([128, OUTP], F32, tag=f"psb{b}", name=f"psb{b}")
                      for b in range(BL // 128)]
            for m in range(OUTP // 128):
                for bt in range(BL // 128):
                    tr = ps.tile([128, 128], F32, tag="pstr", bufs=2)
                    nc.tensor.transpose(tr[:], predT_tiles[m][:, bt * 128:(bt + 1) * 128],
                                        idn[:])
                    nc.scalar.activation(predsb[bt][:, m * 128:(m + 1) * 128], tr[:],
                                         AF.Copy)
            for bt in range(BL // 128):
                nc.sync.dma_start(pred_out[bt * 128:(bt + 1) * 128, :],
                                  predsb[bt][:, 0:OUT])

            # softmax / CE partials per batch tile
            for bt in range(BL // 128):
                negmax = sb.tile([128, 1], F32, tag="negmax", bufs=2)
                nc.vector.tensor_reduce(negmax[:], predsb[bt][:], AX, AluOpType.max,
                                        negate=True)
                je = sb.tile([128, OUTP], F16, tag="je", bufs=2)
                sumexp = sb.tile([128, 1], F32, tag="sumexp", bufs=2)
                nc.scalar.activation(je[:], predsb[bt][:], AF.Exp, bias=negmax[:, 0:1],
                                     accum_out=sumexp[:])
                lse = sb.tile([128, 1], F32, tag="lse", bufs=2)
                nc.scalar.activation(lse[:], sumexp[:], AF.Ln)
                iseq = sb.tile([128, OUTP], F16, tag="iseq", bufs=2)
                nc.vector.tensor_scalar(iseq[:], arbc[:], y_t[:, bt:bt + 1], None,
                                        AluOpType.is_equal)
                jp = sb.tile([128, OUTP], F32, tag="jp", bufs=2)
                py = sb.tile([128, 1], F32, tag="py", bufs=2)
                nc.vector.affine_mul_reduce(jp[:], py[:], iseq[:], predsb[bt][:],
                                            1.0, 0.0)
                tmp = sb.tile([128, 1], F32, tag="cetmp", bufs=2)
                nc.vector.tensor_tensor(tmp[:], py[:], negmax[:], AluOpType.add)
                nc.vector.tensor_tensor(parts_t[:, 8 + bt:9 + bt], tmp[:], lse[:],
                                        AluOpType.subtract)

            # final partial columns
            nc.vector.tensor_reduce(parts_t[0:1, 0:1], ps_w[:], AX, AluOpType.add)
            nc.vector.tensor_reduce(parts_t[:, 1:2], expacc[:], AX, AluOpType.add)
            nc.sync.dma_start(partials[:, :], parts_t[:])

    nc.compile()
    return nc


def _prep(inputs):
    f16, f32 = np.float16, np.float32

    def pack_layer(mu, ls, eps, rank, G, nk, pad_to=None):
        # mu/ls/eps [O, I] -> transposed [I, O], rank half of O (or of I for
        # contraction-sharded layers handled by caller), blocks [G, nk, 128, 1536]
        def blocks(a):
            # a: [I, Ohalf] f16; I = nk*128, Ohalf = G*512
            return a.reshape(nk, 128, G, 512).transpose(2, 0, 1, 3)
        out = np.concatenate([blocks(mu), blocks(ls), blocks(eps)], axis=3)
        return np.ascontiguousarray(out).reshape(G * nk * 128, 1536)

    packs = {}
    for r in range(TP):
        # layer 0: shard output axis H0
        sl = slice(r * H0H, (r + 1) * H0H)
        w0m = inputs["w_mu0"].T[:, sl].astype(f16)
        w0l = inputs["w_ls0"].T[:, sl].astype(f16)
        w0e = inputs["eps_w0"].T[:, sl].astype(f16)
        packs[("wp0", r)] = pack_layer(w0m, w0l, w0e, r, G0, NK_IN)
        # layer 1: shard contraction axis H0, keep full H1
        w1m = inputs["w_mu1"].T[sl, :].astype(f16)
        w1l = inputs["w_ls1"].T[sl, :].astype(f16)
        w1e = inputs["eps_w1"].T[sl, :].astype(f16)
        packs[("wp1", r)] = pack_layer(w1m, w1l, w1e, r, G1, NK_H)
        # output layer: shard contraction axis H1, pad classes to 1024
        slo = slice(r * H1H, (r + 1) * H1H)

        def padded(a):
            out = np.zeros((H1H, OUTP), f16)
            out[:, :OUT] = a.T[slo, :].astype(f16)
            return out
        packs[("wpo", r)] = pack_layer(padded(inputs["w_muo"]),
                                       padded(inputs["w_lso"]),
                                       padded(inputs["eps_wo"]), r, GO, NK_H)

    def vec128(v, ncol):
        return np.ascontiguousarray(v.reshape(ncol, 128).T.astype(f32))

    bvs = {}
    for r in range(TP):
        for li, nm, n in ((0, "0", 16), (1, "1", 16)):
            sl = slice(r * 2048, (r + 1) * 2048)
            bvs[(f"bv{nm}", r)] = np.concatenate(
                [vec128(inputs["b_mu" + nm][sl], 16),
                 vec128(inputs["b_ls" + nm][sl], 16),
                 vec128(inputs["eps_b" + nm][sl], 16)], axis=1)
        bmo = np.full(OUTP, -1e30, f32); bmo[:OUT] = inputs["b_muo"]
        blo = np.zeros(OUTP, f32); blo[:OUT] = inputs["b_lso"]
        beo = np.zeros(OUTP, f32); beo[:OUT] = inputs["eps_bo"]
        bvs[("bvo", r)] = np.concatenate(
            [vec128(bmo, 8), vec128(blo, 8), vec128(beo, 8)], axis=1)
        bvs[("mk0", r)] = vec128(inputs["mask_mu0"][sl].astype(f32), 16)
        bvs[("mk1", r)] = vec128(inputs["mask_mu1"][sl].astype(f32), 16)

    zw2 = vec128(inputs["z_w"], 16)
    ar1 = np.arange(OUTP, dtype=f32)[None, :]
    p128a = np.arange(128, dtype=f32)[:, None]

    xTf = inputs["x"].T.astype(f16)          # [2048, 1024]
    pf0T = inputs["pfz0"].T.astype(f16)      # [4096, 1024]
    pf1T = inputs["pfz1"].T.astype(f16)
    mz0T = inputs["mz0"].T.astype(f16)
    mz1T = inputs["mz1"].T.astype(f16)

    in_maps = []
    for c in range(8):
        p, r = c // 2, c % 2
        bsl = slice(p * BL, (p + 1) * BL)
        hsl = slice(r * 2048, (r + 1) * 2048)
        m = {
            "wp0": packs[("wp0", r)], "wp1": packs[("wp1", r)],
            "wpo": packs[("wpo", r)],
            "xT": np.ascontiguousarray(xTf[:, bsl]),
            "pf0": np.ascontiguousarray(pf0T[hsl, bsl]),
            "pf1": np.ascontiguousarray(pf1T[hsl, bsl]),
            "mz0": np.ascontiguousarray(mz0T[hsl, bsl]),
            "mz1": np.ascontiguousarray(mz1T[hsl, bsl]),
            "bv0": bvs[("bv0", r)], "bv1": bvs[("bv1", r)],
            "bvo": bvs[("bvo", r)],
            "mk0": bvs[("mk0", r)], "mk1": bvs[("mk1", r)],
            "zw2": zw2,
            "y2": np.ascontiguousarray(
                inputs["y"][bsl].astype(f32).reshape(2, 128).T),
            "ar1": ar1, "p128": p128a,
        }
        in_maps.append(m)
    return in_maps


def kernel(**inputs):
    if "nc" not in _CACHE:
        _CACHE["nc"] = _build()
    nc = _CACHE["nc"]
    in_maps = _prep(inputs)
    res = run_bass_kernel_spmd(nc, in_maps, core_ids=list(range(8)),
                               **_CACHE.get("run_kwargs", {}))
    _CACHE["last_res"] = res
    outs = res.results

    pred = np.concatenate([outs[2 * p]["pred_out"] for p in range(DP)], axis=0)

    s = [outs[c]["partials"].astype(np.float64).sum(axis=0) for c in range(8)]
    wsum = s[0][0] + s[1][0]              # -Σls -0.5Σeps² +0.01Σmu² (w matrices)
    expsum = s[0][1] + s[1][1]            # Σ exp(ls) over both w matrices
    bls = s[0][2] + s[1][2] + s[0][3] + s[1][3]
    beb2 = s[0][4] + s[1][4] + s[0][5] + s[1][5]
    flow = sum(s[c][6] for c in range(8))
    xz = sum(s[c][7] for c in (0, 2, 4, 6))
    ce_raw = sum(s[c][8] + s[c][9] for c in (0, 2, 4, 6))

    n_w = H0 * IN + H1 * H0
    n_b = H0 + H1
    L = (xz + flow
         + n_w * C0 + wsum
         + n_b * C0 - bls - 0.5 * beb2
         + 60000.0 * (-ce_raw / B)
         + DECAY * expsum)
    loss = np.float32(np.float32(L) ** 2)
    return pred, loss


# revision 18
# speedup vs baseline: 77.9627x; 77.9627x over previous
"""Bass/Trainium2 kernel for nn_BaysianMLPMaskedDropout (8 NeuronCores).

Sharding: 2 data-parallel groups (batch 1024 -> 2 x 512) x 4-way tensor
parallel (the 4096 hidden axis of each BayesLinear split in quarters).
Activations are kept feature-major ("transposed", [feature, batch]) on chip so
every matmul contracts along the partition axis with no transposes between
layers.  A per-quad ReduceScatter after layer 1 and AllReduce after the output
layer run on the collective engine.  The scalar flow-loss pieces are reduced
on-device into per-core partial vectors and combined on the host.

log(pw) = log(cdf(w+eps)-cdf(w-eps)) is evaluated with the exact expansion
log(2*EPS/sqrt(2pi)) - w_ls - eps^2/2 (error < 1e-7, far below the f32 noise
of the reference's catastrophically-cancelling cdf difference).
"""

import numpy as np

import concourse.bacc as bacc
import concourse.tile as tile
from concourse import mybir
from concourse.alu_op_type import AluOpType
from concourse.bass_utils import run_bass_kernel_spmd

F16 = mybir.dt.float16
F32 = mybir.dt.float32
AF = mybir.ActivationFunctionType
AX = mybir.AxisListType.X

B, IN, H0, H1, OUT = 1024, 2048, 4096, 4096, 1000
OUTP = 1024                      # classes padded to 8*128 for uniform tiles
TP, DP = 4, 2
BL = B // DP                     # batch rows per DP group
NBT = BL // 128                  # batch tiles (softmax / transpose)
H0H, H1H = H0 // TP, H1 // TP    # per-rank hidden slice
NK_IN = IN // 128                # contraction chunks, layer 0
NK_H = H0H // 128                # contraction chunks, layers 1/o
G0, G1, GO = H0H // 512, H1 // 512, OUTP // 512
NBLK = G0 * NK_IN + G1 * NK_H    # weight-stream blocks with flow sums
EPS_CDF = 1e-5
C0 = float(np.log(2 * EPS_CDF / np.sqrt(2 * np.pi)))
NEG = 0.01
DECAY = 0.01
CLIP_LO = float(np.float32(1e-6))
CLIP_HI = float(np.float32(1.0 - 1e-6))

_CACHE = {}


def _build():
    nc = bacc.Bacc("TRN2", debug=False, num_devices=8, enable_asserts=False)

    def din(name, shape, dt=F16):
        return nc.dram_tensor(name, shape, dt, kind="ExternalInput").ap()

    wp0 = din("wp0", [G0 * NK_IN * 128, 1536])
    wp1 = din("wp1", [G1 * NK_H * 128, 1536])
    wpo = din("wpo", [GO * NK_H * 128, 1536])
    xT = din("xT", [IN, BL])
    pf0 = din("pf0", [H0H, BL])
    pf1 = din("pf1", [H1H, BL])
    mz0 = din("mz0", [H0H, BL])
    mz1 = din("mz1", [H1H, BL])
    NC = H0H // 128              # bias columns per layer slice
    bv0 = din("bv0", [128, 3 * NC], F32)   # [mu | ls | eps]
    bv1 = din("bv1", [128, 3 * NC], F32)
    bvo = din("bvo", [128, 24], F32)       # padded to 1024 classes
    mk0 = din("mk0", [128, NC], F32)
    mk1 = din("mk1", [128, NC], F32)
    zw2 = din("zw2", [128, NK_IN])         # f16, pre-scaled by 1/TP
    y2 = din("y2", [128, NBT], F32)
    ar1 = din("ar1", [1, OUTP], F32)
    p128 = din("p128", [128, 1], F32)

    pred_out = nc.dram_tensor("pred_out", [BL, OUT], F32, kind="ExternalOutput").ap()
    partials = nc.dram_tensor("partials", [128, 16], F32, kind="ExternalOutput").ap()

    groups = [list(range(i, i + TP)) for i in range(0, 8, TP)]

    with tile.TileContext(nc) as tc:
        with (
            tc.tile_pool(name="sb", bufs=1) as sb,
            tc.tile_pool(name="ps", bufs=1, space="PSUM") as ps,
            tc.tile_pool(name="dram", bufs=1, space="DRAM") as dram,
        ):
            # ---------------- small constant loads ----------------
            def load(name, src, shape, dt=F32):
                t = sb.tile(shape, dt, tag=name)
                nc.sync.dma_start(t[:], src)
                return t

            bv0_t = load("bv0", bv0[:, :], [128, 3 * NC])
            bv1_t = load("bv1", bv1[:, :], [128, 3 * NC])
            bvo_t = load("bvo", bvo[:, :], [128, 24])
            mk0_t = load("mk0", mk0[:, :], [128, NC])
            mk1_t = load("mk1", mk1[:, :], [128, NC])
            zw_t = load("zw", zw2[:, :], [128, NK_IN], F16)
            y_t = load("y", y2[:, :], [128, NBT])
            p128_t = load("p128", p128[:, :], [128, 1])
            ar_row = sb.tile([1, OUTP], F32, tag="ar_row")
            nc.sync.dma_start(ar_row[:], ar1[0:1, :])
            arbc = sb.tile([128, OUTP], F32, tag="arbc")
            nc.gpsimd.partition_broadcast(arbc[:], ar_row[:])
            # f16 copies of the mask columns (used as matmul lhsT for msum)
            mk0h = sb.tile([128, NC], F16, tag="mk0h")
            nc.vector.tensor_copy(mk0h[:], mk0_t[:])
            mk1h = sb.tile([128, NC], F16, tag="mk1h")
            nc.vector.tensor_copy(mk1h[:], mk1_t[:])

            negones_t = sb.tile([128, 1], F16, tag="negones")
            nc.vector.memset(negones_t[:], -1.0)

            parts_t = sb.tile([128, 16], F32, tag="parts")
            nc.vector.memset(parts_t[:], 0.0)

            # bias materialization: b = b_mu + exp(b_ls) * b_eps
            def bias_mat(bv, ncol, tag):
                s = sb.tile([128, ncol], F32, tag=tag + "s")
                nc.scalar.activation(s[:], bv[:, ncol:2 * ncol], AF.Exp)
                t = sb.tile([128, ncol], F32, tag=tag + "t")
                nc.vector.tensor_tensor(t[:], s[:], bv[:, 2 * ncol:3 * ncol], AluOpType.mult)
                b = sb.tile([128, ncol], F32, tag=tag + "b")
                nc.vector.tensor_tensor(b[:], t[:], bv[:, 0:ncol], AluOpType.add)
                return b

            b0m = bias_mat(bv0_t, NC, "b0")
            b1m = bias_mat(bv1_t, NC, "b1")
            bom = bias_mat(bvo_t, 8, "bo")

            # b-vector flow partials: sum(b_ls), sum(b_eps^2) for layers 0,1
            nc.vector.tensor_reduce(parts_t[:, 2:3], bv0_t[:, NC:2 * NC], AX, AluOpType.add)
            nc.vector.tensor_reduce(parts_t[:, 3:4], bv1_t[:, NC:2 * NC], AX, AluOpType.add)
            jb = sb.tile([128, NC], F32, tag="jb", bufs=2)
            nc.vector.affine_mul_reduce(jb[:], parts_t[:, 4:5], bv0_t[:, 2 * NC:3 * NC],
                                        bv0_t[:, 2 * NC:3 * NC], 1.0, 0.0)
            jb2 = sb.tile([128, NC], F32, tag="jb", bufs=2)
            nc.vector.affine_mul_reduce(jb2[:], parts_t[:, 5:6], bv1_t[:, 2 * NC:3 * NC],
                                        bv1_t[:, 2 * NC:3 * NC], 1.0, 0.0)

            # ---------------- x load ----------------
            xall = sb.tile([128, NK_IN * BL], F16, tag="xall")
            for k in range(NK_IN):
                nc.sync.dma_start(xall[:, k * BL:(k + 1) * BL],
                                  xT[k * 128:(k + 1) * 128, :])

            # combined scalar PSUM row: -sum(ls) + 0.01*sum(mu^2 via DVE? no –
            # ls via neg-ones matmuls) + xz/TP via zw-column matmuls
            ps_w = ps.tile([1, 512], F32, tag="psw")
            n_psw = NBLK + NK_IN   # total accumulating matmuls into ps_w
            psw_i = [0]

            def psw_mm(lhsT, rhs):
                nc.tensor.matmul(ps_w[:], lhsT, rhs, start=(psw_i[0] == 0),
                                 stop=(psw_i[0] == n_psw - 1), skip_group_check=True)
                psw_i[0] += 1

            # xz/TP partial: zw columns (pre-scaled 1/TP) against x chunks
            for k in range(NK_IN):
                psw_mm(zw_t[:, k:k + 1], xall[:, k * BL:(k + 1) * BL])

            # ---------------- masks + pfz flow (both layers) ----------------
            flacc = sb.tile([128, 4 * NK_H], F32, tag="flacc")
            ps_ms = ps.tile([1, BL], F32, tag="psms")
            ms_sb = sb.tile([1, 2 * BL], F32, tag="ms_sb")

            def mask_flow(mzap, pfap, mkh, layer):
                mzall = sb.tile([128, NK_H * BL], F16, tag=f"mzall{layer}",
                                name=f"mzall{layer}")
                pfall = sb.tile([128, NK_H * BL], F16, tag="pfall",
                                name=f"pfall{layer}")
                for k in range(NK_H):
                    nc.sync.dma_start(mzall[:, k * BL:(k + 1) * BL],
                                      mzap[k * 128:(k + 1) * 128, :])
                    nc.sync.dma_start(pfall[:, k * BL:(k + 1) * BL],
                                      pfap[k * 128:(k + 1) * 128, :])
                for k in range(NK_H):
                    mzs = mzall[:, k * BL:(k + 1) * BL]
                    pc = sb.tile([128, BL], F32, tag="pc", bufs=3)
                    nc.vector.tensor_scalar(pc[:], pfall[:, k * BL:(k + 1) * BL],
                                            CLIP_LO, CLIP_HI,
                                            AluOpType.max, AluOpType.min)
                    lnp = sb.tile([128, BL], F32, tag="lnp", bufs=3)
                    nc.scalar.activation(lnp[:], pc[:], AF.Ln)
                    ln1p = sb.tile([128, BL], F32, tag="ln1p", bufs=3)
                    nc.scalar.activation(ln1p[:], pc[:], AF.Ln, bias=1.0, scale=-1.0,
                                         accum_out=flacc[:, layer * 2 * NK_H + k:
                                                         layer * 2 * NK_H + k + 1])
                    d = sb.tile([128, BL], F32, tag="dln", bufs=3)
                    nc.vector.tensor_tensor(d[:], lnp[:], ln1p[:], AluOpType.subtract)
                    jf = sb.tile([128, BL], F32, tag="jf", bufs=2)
                    nc.vector.affine_mul_reduce(
                        jf[:], flacc[:, layer * 2 * NK_H + NK_H + k:
                                     layer * 2 * NK_H + NK_H + k + 1],
                        mzs, d[:], 1.0, 0.0)
                    # msum partial: mask column as stationary against raw mz
                    nc.tensor.matmul(ps_ms[:], mkh[:, k:k + 1], mzs,
                                     start=(k == 0), stop=(k == NK_H - 1))
                nc.vector.tensor_copy(ms_sb[:, layer * BL:(layer + 1) * BL], ps_ms[:])
                return mzall

            mz0all = mask_flow(mz0[:, :], pf0[:, :], mk0h, 0)
            mz1all = mask_flow(mz1[:, :], pf1[:, :], mk1h, 1)
            nc.vector.tensor_reduce(parts_t[:, 6:7], flacc[:, :], AX, AluOpType.add)

            # msum AllReduce (quad) -> mult factors broadcast to 128 partitions
            cc_ms_in = dram.tile([1, 2 * BL], F32)
            cc_ms_out = dram.tile([1, 2 * BL], F32)
            nc.sync.dma_start(cc_ms_in[:], ms_sb[:])
            nc.gpsimd.collective_compute("AllReduce", AluOpType.add,
                                         replica_groups=groups,
                                         ins=[cc_ms_in.opt()], outs=[cc_ms_out.opt()])
            ms_row = sb.tile([1, 2 * BL], F32, tag="ms_row")
            nc.sync.dma_start(ms_row[:], cc_ms_out[0:1, :])
            multbc = sb.tile([128, 2 * BL], F32, tag="multbc")
            nc.gpsimd.partition_broadcast(multbc[:], ms_row[:])
            nc.vector.tensor_scalar(multbc[:], multbc[:], 1e-6, None, AluOpType.add)
            nc.vector.reciprocal(multbc[:], multbc[:])
            nc.vector.tensor_scalar(multbc[:], multbc[:], float(H0), None, AluOpType.mult)
            mult0bc = multbc[:, 0:BL]
            mult1bc = multbc[:, BL:2 * BL]

            # ---------------- streaming weight layers ----------------
            expacc = sb.tile([128, NBLK], F32, tag="expacc")
            wacc = sb.tile([128, 2 * NBLK], F32, tag="wacc")

            def wstream(wp, G, nk, rhs_sl, with_sums, blk0, evict):
                for g in range(G):
                    pts = [ps.tile([128, BL], F32, tag="psmain", bufs=5,
                                   name=f"psm{blk0}_{g}_{m}_{id(evict)}") for m in range(4)]
                    for k in range(nk):
                        blk = sb.tile([128, 1536], F16, tag="wblk", bufs=3)
                        row = (g * nk + k) * 128
                        eng = nc.sync if (k % 2 == 0) else nc.scalar
                        eng.dma_start(blk[:], wp[row:row + 128, :])
                        mu, ls, eps = blk[:, 0:512], blk[:, 512:1024], blk[:, 1024:1536]
                        s_t = sb.tile([128, 512], F16, tag="s", bufs=3)
                        bi = blk0 + g * nk + k
                        nc.scalar.activation(
                            s_t[:], ls, AF.Exp,
                            accum_out=(expacc[:, bi:bi + 1] if with_sums else None))
                        t_t = sb.tile([128, 512], F16, tag="t", bufs=3)
                        nc.vector.tensor_tensor(t_t[:], s_t[:], eps, AluOpType.mult)
                        w_t = sb.tile([128, 512], F16, tag="w", bufs=3)
                        nc.gpsimd.tensor_tensor(w_t[:], t_t[:], mu, AluOpType.add)
                        if with_sums:
                            psw_mm(negones_t[:], ls)            # -sum(ls)
                            j1 = sb.tile([128, 512], F16, tag="j1", bufs=2)
                            nc.vector.scalar_tensor_tensor(
                                j1[:], eps, -0.5, eps, AluOpType.mult, AluOpType.mult,
                                accum_out=wacc[:, 2 * bi:2 * bi + 1])
                            j2 = sb.tile([128, 512], F16, tag="j2", bufs=2)
                            nc.vector.scalar_tensor_tensor(
                                j2[:], mu, DECAY, mu, AluOpType.mult, AluOpType.mult,
                                accum_out=wacc[:, 2 * bi + 1:2 * bi + 2])
                        for m in range(4):
                            nc.tensor.matmul(pts[m][:], w_t[:, m * 128:(m + 1) * 128],
                                             rhs_sl(k), start=(k == 0), stop=(k == nk - 1))
                    for m in range(4):
                        evict(g * 4 + m, pts[m])

            # ---- layer 0 ----
            h0m_tiles = [None] * NK_H

            def evict0(j, pt):
                lr = sb.tile([128, BL], F32, tag="h0lr", bufs=2)
                nc.scalar.activation(lr[:], pt[:], AF.Lrelu, bias=b0m[:, j:j + 1],
                                     alpha=NEG)
                hm = sb.tile([128, BL], F16, tag=f"h0m{j}", name=f"h0m{j}")
                nc.vector.scalar_tensor_tensor(hm[:], lr[:], mk0_t[:, j:j + 1],
                                               mz0all[:, j * BL:(j + 1) * BL],
                                               AluOpType.mult, AluOpType.mult)
                h0m_tiles[j] = hm

            wstream(wp0, G0, NK_IN, lambda k: xall[:, k * BL:(k + 1) * BL],
                    True, 0, evict0)

            # ---- layer 1 (partials over my H0 slice, full H1) ----
            cc_h1in = dram.tile([H1, BL], F16)
            cc_h1rs = dram.tile([H1H, BL], F16)

            def evict1(j, pt):
                hp = sb.tile([128, BL], F16, tag="h1p", bufs=2)
                nc.vector.tensor_copy(hp[:], pt[:])
                nc.gpsimd.dma_start(cc_h1in[j * 128:(j + 1) * 128, :], hp[:])

            wstream(wp1, G1, NK_H, lambda k: h0m_tiles[k][:],
                    True, G0 * NK_IN, evict1)

            nc.gpsimd.collective_compute("ReduceScatter", AluOpType.add,
                                         replica_groups=groups,
                                         ins=[cc_h1in.opt()], outs=[cc_h1rs.opt()])

            h1s = sb.tile([128, NK_H * BL], F16, tag="h1s")
            for k in range(NK_H):
                nc.sync.dma_start(h1s[:, k * BL:(k + 1) * BL],
                                  cc_h1rs[k * 128:(k + 1) * 128, :])
            h1m_tiles = []
            for k in range(NK_H):
                a = sb.tile([128, BL], F32, tag="h1a", bufs=2)
                nc.vector.tensor_tensor(a[:], h1s[:, k * BL:(k + 1) * BL], mult0bc,
                                        AluOpType.mult)
                lr = sb.tile([128, BL], F32, tag="h1lr", bufs=2)
                nc.scalar.activation(lr[:], a[:], AF.Lrelu, bias=b1m[:, k:k + 1],
                                     alpha=NEG)
                hm = sb.tile([128, BL], F16, tag=f"h1m{k}", name=f"h1m{k}")
                nc.vector.scalar_tensor_tensor(hm[:], lr[:], mk1_t[:, k:k + 1],
                                               mz1all[:, k * BL:(k + 1) * BL],
                                               AluOpType.mult, AluOpType.mult)
                h1m_tiles.append(hm)

            # ---- output layer ----
            cc_pin = dram.tile([OUTP, BL], F16)
            cc_pout = dram.tile([OUTP, BL], F16)

            def evicto(j, pt):
                pp = sb.tile([128, BL], F16, tag="pp", bufs=2)
                nc.scalar.activation(pp[:], pt[:], AF.Copy)
                nc.gpsimd.dma_start(cc_pin[j * 128:(j + 1) * 128, :], pp[:])

            wstream(wpo, GO, NK_H, lambda k: h1m_tiles[k][:],
                    False, 0, evicto)

            nc.gpsimd.collective_compute("AllReduce", AluOpType.add,
                                         replica_groups=groups,
                                         ins=[cc_pin.opt()], outs=[cc_pout.opt()])

            par = sb.tile([128, (OUTP // 128) * BL], F16, tag="xall", name="par")
            for m in range(OUTP // 128):
                nc.sync.dma_start(par[:, m * BL:(m + 1) * BL],
                                  cc_pout[m * 128:(m + 1) * 128, :])
            predT_tiles = []
            for m in range(OUTP // 128):
                pm = sb.tile([128, BL], F32, tag="pTa", bufs=2)
                nc.vector.tensor_tensor(pm[:], par[:, m * BL:(m + 1) * BL], mult1bc,
                                        AluOpType.mult)
                pt2 = sb.tile([128, BL], F32, tag=f"pT{m}", name=f"pT{m}")
                nc.vector.tensor_scalar(pt2[:], pm[:], bom[:, m:m + 1], None,
                                        AluOpType.add)
                predT_tiles.append(pt2)

            # transpose predT [1024, BL] -> pred [BL, 1024]
            idn = sb.tile([128, 128], F32, tag="idn")
            nc.vector.tensor_scalar(idn[:], arbc[:, 0:128], p128_t[:, 0:1], None,
                                    AluOpType.is_equal)
            # softmax / CE partials per batch tile (2 rotating pred buffers)
            ceacc = sb.tile([128, NBT], F32, tag="ceacc")
            for bt in range(NBT):
                psb = sb.tile([128, OUTP], F32, tag="psb", bufs=2, name=f"psb{bt}")
                for m in range(OUTP // 128):
                    tr = ps.tile([128, 128], F32, tag="pstr", bufs=1,
                                 name=f"tr{m}_{bt}")
                    nc.tensor.transpose(tr[:], predT_tiles[m][:, bt * 128:(bt + 1) * 128],
                                        idn[:])
                    nc.scalar.activation(psb[:, m * 128:(m + 1) * 128], tr[:],
                                         AF.Copy)
                nc.sync.dma_start(pred_out[bt * 128:(bt + 1) * 128, :],
                                  psb[:, 0:OUT])
                negmax = sb.tile([128, 1], F32, tag="negmax", bufs=2)
                nc.vector.tensor_reduce(negmax[:], psb[:], AX, AluOpType.max,
                                        negate=True)
                je = sb.tile([128, OUTP], F16, tag="je", bufs=1)
                sumexp = sb.tile([128, 1], F32, tag="sumexp", bufs=2)
                nc.scalar.activation(je[:], psb[:], AF.Exp, bias=negmax[:, 0:1],
                                     accum_out=sumexp[:])
                lse = sb.tile([128, 1], F32, tag="lse", bufs=2)
                nc.scalar.activation(lse[:], sumexp[:], AF.Ln)
                iseq = sb.tile([128, OUTP], F16, tag="iseq", bufs=1)
                nc.vector.tensor_scalar(iseq[:], arbc[:], y_t[:, bt:bt + 1], None,
                                        AluOpType.is_equal)
                jp = sb.tile([128, OUTP], F32, tag="jp", bufs=1)
                py = sb.tile([128, 1], F32, tag="py", bufs=2)
                nc.vector.affine_mul_reduce(jp[:], py[:], iseq[:], psb[:],
                                            1.0, 0.0)
                tmp = sb.tile([128, 1], F32, tag="cetmp", bufs=2)
                nc.vector.tensor_tensor(tmp[:], py[:], negmax[:], AluOpType.add)
                nc.vector.tensor_tensor(ceacc[:, bt:bt + 1], tmp[:], lse[:],
                                        AluOpType.subtract)
            nc.vector.tensor_reduce(parts_t[:, 8:9], ceacc[:, :], AX, AluOpType.add)

            # final partial columns
            nc.vector.tensor_reduce(parts_t[0:1, 0:1], ps_w[:], AX, AluOpType.add)
            nc.vector.tensor_reduce(parts_t[:, 1:2], expacc[:], AX, AluOpType.add)
            nc.vector.tensor_reduce(parts_t[:, 9:10], wacc[:], AX, AluOpType.add)
            nc.sync.dma_start(partials[:, :], parts_t[:])

    nc.compile()
    return nc


def _prep(inputs):
    f16, f32 = np.float16, np.float32

    def pack_layer(mu, ls, eps, G, nk):
        # arrays [I_local, O_local] f16 -> [G*nk*128, 1536] block-contiguous
        def blocks(a):
            return a.reshape(nk, 128, G, 512).transpose(2, 0, 1, 3)
        out = np.concatenate([blocks(mu), blocks(ls), blocks(eps)], axis=3)
        return np.ascontiguousarray(out).reshape(G * nk * 128, 1536)

    packs = {}
    for r in range(TP):
        sl0 = slice(r * H0H, (r + 1) * H0H)
        packs[("wp0", r)] = pack_layer(inputs["w_mu0"].T[:, sl0].astype(f16),
                                       inputs["w_ls0"].T[:, sl0].astype(f16),
                                       inputs["eps_w0"].T[:, sl0].astype(f16),
                                       G0, NK_IN)
        packs[("wp1", r)] = pack_layer(inputs["w_mu1"].T[sl0, :].astype(f16),
                                       inputs["w_ls1"].T[sl0, :].astype(f16),
                                       inputs["eps_w1"].T[sl0, :].astype(f16),
                                       G1, NK_H)
        slo = slice(r * H1H, (r + 1) * H1H)

        def padded(a):
            out = np.zeros((H1H, OUTP), f16)
            out[:, :OUT] = a.T[slo, :].astype(f16)
            return out
        packs[("wpo", r)] = pack_layer(padded(inputs["w_muo"]),
                                       padded(inputs["w_lso"]),
                                       padded(inputs["eps_wo"]), GO, NK_H)

    def vec128(v, ncol):
        return np.ascontiguousarray(v.reshape(ncol, 128).T.astype(f32))

    NC = H0H // 128
    bvs = {}
    for r in range(TP):
        for nm in ("0", "1"):
            sl = slice(r * H0H, (r + 1) * H0H)
            bvs[(f"bv{nm}", r)] = np.concatenate(
                [vec128(inputs["b_mu" + nm][sl], NC),
                 vec128(inputs["b_ls" + nm][sl], NC),
                 vec128(inputs["eps_b" + nm][sl], NC)], axis=1)
        bmo = np.full(OUTP, -1e30, f32); bmo[:OUT] = inputs["b_muo"]
        blo = np.zeros(OUTP, f32); blo[:OUT] = inputs["b_lso"]
        beo = np.zeros(OUTP, f32); beo[:OUT] = inputs["eps_bo"]
        bvs[("bvo", r)] = np.concatenate(
            [vec128(bmo, 8), vec128(blo, 8), vec128(beo, 8)], axis=1)
        bvs[("mk0", r)] = vec128(inputs["mask_mu0"][sl].astype(f32), NC)
        bvs[("mk1", r)] = vec128(inputs["mask_mu1"][sl].astype(f32), NC)

    zw2 = np.ascontiguousarray(
        (inputs["z_w"].astype(f32) / TP).reshape(NK_IN, 128).T.astype(f16))
    ar1 = np.arange(OUTP, dtype=f32)[None, :]
    p128a = np.arange(128, dtype=f32)[:, None]

    xTf = inputs["x"].T.astype(f16)
    pf0T = inputs["pfz0"].T.astype(f16)
    pf1T = inputs["pfz1"].T.astype(f16)
    mz0T = inputs["mz0"].T.astype(f16)
    mz1T = inputs["mz1"].T.astype(f16)

    in_maps = []
    for c in range(8):
        p, r = c // TP, c % TP
        bsl = slice(p * BL, (p + 1) * BL)
        hsl = slice(r * H0H, (r + 1) * H0H)
        m = {
            "wp0": packs[("wp0", r)], "wp1": packs[("wp1", r)],
            "wpo": packs[("wpo", r)],
            "xT": np.ascontiguousarray(xTf[:, bsl]),
            "pf0": np.ascontiguousarray(pf0T[hsl, bsl]),
            "pf1": np.ascontiguousarray(pf1T[hsl, bsl]),
            "mz0": np.ascontiguousarray(mz0T[hsl, bsl]),
            "mz1": np.ascontiguousarray(mz1T[hsl, bsl]),
            "bv0": bvs[("bv0", r)], "bv1": bvs[("bv1", r)],
            "bvo": bvs[("bvo", r)],
            "mk0": bvs[("mk0", r)], "mk1": bvs[("mk1", r)],
            "zw2": zw2,
            "y2": np.ascontiguousarray(
                inputs["y"][bsl].astype(f32).reshape(NBT, 128).T),
            "ar1": ar1, "p128": p128a,
        }
        in_maps.append(m)
    return in_maps


def kernel(**inputs):
    if "nc" not in _CACHE:
        _CACHE["nc"] = _build()
    nc = _CACHE["nc"]
    in_maps = _prep(inputs)
    res = run_bass_kernel_spmd(nc, in_maps, core_ids=list(range(8)),
                               **_CACHE.get("run_kwargs", {}))
    _CACHE["last_res"] = res
    outs = res.results

    pred = np.concatenate([outs[TP * p]["pred_out"] for p in range(DP)], axis=0)

    s = [outs[c]["partials"].astype(np.float64).sum(axis=0) for c in range(8)]
    g0 = range(TP)                       # one DP group's ranks cover all shards
    pairs_dedup = [TP * p for p in range(DP)]
    psw = sum(s[c][0] for c in g0)       # -Σls (w) + xz (zw pre-scaled by 1/TP)
    expsum = sum(s[c][1] for c in g0)    # Σ exp(ls), both w matrices
    bls = sum(s[c][2] + s[c][3] for c in g0)
    beb2 = sum(s[c][4] + s[c][5] for c in g0)
    flow = sum(s[c][6] for c in range(8))
    ce_raw = sum(s[c][8] for c in pairs_dedup)
    wsq = sum(s[c][9] for c in g0)       # -0.5Σeps² + 0.01Σmu²

    n_w = H0 * IN + H1 * H0
    n_b = H0 + H1
    L = (psw + wsq + flow
         + n_w * C0
         + n_b * C0 - bls - 0.5 * beb2
         + 60000.0 * (-ce_raw / B)
         + DECAY * expsum)
    loss = np.float32(np.float32(L) ** 2)
    return pred, loss


# revision 23
# speedup vs baseline: 81.1677x; 1.0411x over previous
"""Bass/Trainium2 kernel for nn_BaysianMLPMaskedDropout (8 NeuronCores).

Sharding: 2 data-parallel groups (batch 1024 -> 2 x 512) x 4-way tensor
parallel (the 4096 hidden axis of each BayesLinear split in quarters).
Activations are kept feature-major ("transposed", [feature, batch]) on chip so
every matmul contracts along the partition axis with no transposes between
layers.  A per-quad ReduceScatter after layer 1 and AllReduce after the output
layer run on the collective engine.  The scalar flow-loss pieces are reduced
on-device into per-core partial vectors and combined on the host.

log(pw) = log(cdf(w+eps)-cdf(w-eps)) is evaluated with the exact expansion
log(2*EPS/sqrt(2pi)) - w_ls - eps^2/2 (error < 1e-7, far below the f32 noise
of the reference's catastrophically-cancelling cdf difference).
"""

import numpy as np

import concourse.bacc as bacc
import concourse.tile as tile
from concourse import hw_specs as _hw_specs

# All ACT functions this kernel uses (Exp, Ln, Lrelu, Copy, Square) co-reside
# in the natural_log_exp_and_others table set; restrict the set chooser to it
# so the scheduler emits one table load instead of ~26 (2.7us each).
_orig_gat = _hw_specs.get_activation_tables

def _gat_one_set(arch):
    tabs = _orig_gat(arch)
    pref = "natural_log_exp_and_others"
    if pref in tabs:
        tabs = {k: (v if k == pref else set()) for k, v in tabs.items()}
    return tabs

bacc.get_activation_tables = _gat_one_set
from concourse import mybir
from concourse.alu_op_type import AluOpType
from concourse.bass_utils import run_bass_kernel_spmd

F16 = mybir.dt.float16
F32 = mybir.dt.float32
AF = mybir.ActivationFunctionType
AX = mybir.AxisListType.X

B, IN, H0, H1, OUT = 1024, 2048, 4096, 4096, 1000
OUTP = 1024                      # classes padded to 8*128 for uniform tiles
TP, DP = 4, 2
BL = B // DP                     # batch rows per DP group
NBT = BL // 128                  # batch tiles (softmax / transpose)
H0H, H1H = H0 // TP, H1 // TP    # per-rank hidden slice
NK_IN = IN // 128                # contraction chunks, layer 0
NK_H = H0H // 128                # contraction chunks, layers 1/o
G0, G1, GO = H0H // 512, H1 // 512, OUTP // 512
NBLK = G0 * NK_IN + G1 * NK_H    # weight-stream blocks with flow sums
EPS_CDF = 1e-5
C0 = float(np.log(2 * EPS_CDF / np.sqrt(2 * np.pi)))
NEG = 0.01
DECAY = 0.01
CLIP_LO = float(np.float32(1e-6))
CLIP_HI = float(np.float32(1.0 - 1e-6))

_CACHE = {}


def _build():
    nc = bacc.Bacc("TRN2", debug=False, num_devices=8, enable_asserts=False)

    def din(name, shape, dt=F16):
        return nc.dram_tensor(name, shape, dt, kind="ExternalInput").ap()

    wp0 = din("wp0", [G0 * NK_IN * 128, 1536])
    wp1 = din("wp1", [G1 * NK_H * 128, 1536])
    wpo = din("wpo", [GO * NK_H * 128, 1536])
    xT = din("xT", [IN, BL])
    pf0 = din("pf0", [H0H, BL])
    pf1 = din("pf1", [H1H, BL])
    mz0 = din("mz0", [H0H, BL])
    mz1 = din("mz1", [H1H, BL])
    NC = H0H // 128              # bias columns per layer slice
    bv0 = din("bv0", [128, 3 * NC], F32)   # [mu | ls | eps]
    bv1 = din("bv1", [128, 3 * NC], F32)
    bvo = din("bvo", [128, 24], F32)       # padded to 1024 classes
    mk0 = din("mk0", [128, NC], F32)
    mk1 = din("mk1", [128, NC], F32)
    zw2 = din("zw2", [128, NK_IN])         # f16, pre-scaled by 1/TP
    y2 = din("y2", [128, NBT], F32)
    ar1 = din("ar1", [1, OUTP], F32)
    p128 = din("p128", [128, 1], F32)

    pred_out = nc.dram_tensor("pred_out", [BL, OUT], F32, kind="ExternalOutput").ap()
    partials = nc.dram_tensor("partials", [128, 16], F32, kind="ExternalOutput").ap()

    groups = [list(range(i, i + TP)) for i in range(0, 8, TP)]

    with tile.TileContext(nc) as tc:
        with (
            tc.tile_pool(name="sb", bufs=1) as sb,
            tc.tile_pool(name="ps", bufs=1, space="PSUM") as ps,
            tc.tile_pool(name="dram", bufs=1, space="DRAM") as dram,
        ):
            # ---------------- small constant loads ----------------
            def load(name, src, shape, dt=F32):
                t = sb.tile(shape, dt, tag=name)
                nc.sync.dma_start(t[:], src)
                return t

            bv0_t = load("bv0", bv0[:, :], [128, 3 * NC])
            bv1_t = load("bv1", bv1[:, :], [128, 3 * NC])
            bvo_t = load("bvo", bvo[:, :], [128, 24])
            mk0_t = load("mk0", mk0[:, :], [128, NC])
            mk1_t = load("mk1", mk1[:, :], [128, NC])
            zw_t = load("zw", zw2[:, :], [128, NK_IN], F16)
            y_t = load("y", y2[:, :], [128, NBT])
            p128_t = load("p128", p128[:, :], [128, 1])
            ar_row = sb.tile([1, OUTP], F32, tag="ar_row")
            nc.sync.dma_start(ar_row[:], ar1[0:1, :])
            arbc = sb.tile([128, OUTP], F32, tag="arbc")
            nc.gpsimd.partition_broadcast(arbc[:], ar_row[:])
            # f16 copies of the mask columns (used as matmul lhsT for msum)
            mk0h = sb.tile([128, NC], F16, tag="mk0h")
            nc.vector.tensor_copy(mk0h[:], mk0_t[:])
            mk1h = sb.tile([128, NC], F16, tag="mk1h")
            nc.vector.tensor_copy(mk1h[:], mk1_t[:])

            negones_t = sb.tile([128, 1], F16, tag="negones")
            nc.vector.memset(negones_t[:], -1.0)

            parts_t = sb.tile([128, 16], F32, tag="parts")
            nc.vector.memset(parts_t[:], 0.0)

            # bias materialization: b = b_mu + exp(b_ls) * b_eps
            def bias_mat(bv, ncol, tag):
                s = sb.tile([128, ncol], F32, tag=tag + "s")
                nc.scalar.activation(s[:], bv[:, ncol:2 * ncol], AF.Exp)
                t = sb.tile([128, ncol], F32, tag=tag + "t")
                nc.vector.tensor_tensor(t[:], s[:], bv[:, 2 * ncol:3 * ncol], AluOpType.mult)
                b = sb.tile([128, ncol], F32, tag=tag + "b")
                nc.vector.tensor_tensor(b[:], t[:], bv[:, 0:ncol], AluOpType.add)
                return b

            b0m = bias_mat(bv0_t, NC, "b0")
            b1m = bias_mat(bv1_t, NC, "b1")
            bom = bias_mat(bvo_t, 8, "bo")

            # b-vector flow partials: sum(b_ls), sum(b_eps^2) for layers 0,1
            nc.vector.tensor_reduce(parts_t[:, 2:3], bv0_t[:, NC:2 * NC], AX, AluOpType.add)
            nc.vector.tensor_reduce(parts_t[:, 3:4], bv1_t[:, NC:2 * NC], AX, AluOpType.add)
            jb = sb.tile([128, NC], F32, tag="jb", bufs=2)
            nc.vector.affine_mul_reduce(jb[:], parts_t[:, 4:5], bv0_t[:, 2 * NC:3 * NC],
                                        bv0_t[:, 2 * NC:3 * NC], 1.0, 0.0)
            jb2 = sb.tile([128, NC], F32, tag="jb", bufs=2)
            nc.vector.affine_mul_reduce(jb2[:], parts_t[:, 5:6], bv1_t[:, 2 * NC:3 * NC],
                                        bv1_t[:, 2 * NC:3 * NC], 1.0, 0.0)

            # ---------------- x load ----------------
            xall = sb.tile([128, NK_IN * BL], F16, tag="xall")
            for k in range(NK_IN):
                nc.sync.dma_start(xall[:, k * BL:(k + 1) * BL],
                                  xT[k * 128:(k + 1) * 128, :])

            # combined scalar PSUM row: -sum(ls) + 0.01*sum(mu^2 via DVE? no –
            # ls via neg-ones matmuls) + xz/TP via zw-column matmuls
            ps_w = ps.tile([1, 512], F32, tag="psw")
            n_psw = NBLK + NK_IN   # total accumulating matmuls into ps_w
            psw_i = [0]

            def psw_mm(lhsT, rhs):
                nc.tensor.matmul(ps_w[:], lhsT, rhs, start=(psw_i[0] == 0),
                                 stop=(psw_i[0] == n_psw - 1), skip_group_check=True)
                psw_i[0] += 1

            # xz/TP partial: zw columns (pre-scaled 1/TP) against x chunks
            for k in range(NK_IN):
                psw_mm(zw_t[:, k:k + 1], xall[:, k * BL:(k + 1) * BL])

            # ---------------- masks + pfz flow (both layers) ----------------
            flacc = sb.tile([128, 4 * NK_H], F32, tag="flacc")
            ps_ms = ps.tile([1, BL], F32, tag="psms")
            ms_sb = sb.tile([1, 2 * BL], F32, tag="ms_sb")

            def mask_flow(mzap, pfap, mkh, layer):
                mzall = sb.tile([128, NK_H * BL], F16, tag=f"mzall{layer}",
                                name=f"mzall{layer}")
                pfall = sb.tile([128, NK_H * BL], F16, tag="pfall",
                                name=f"pfall{layer}")
                for k in range(NK_H):
                    nc.sync.dma_start(mzall[:, k * BL:(k + 1) * BL],
                                      mzap[k * 128:(k + 1) * 128, :])
                    nc.sync.dma_start(pfall[:, k * BL:(k + 1) * BL],
                                      pfap[k * 128:(k + 1) * 128, :])
                for k in range(NK_H):
                    mzs = mzall[:, k * BL:(k + 1) * BL]
                    pc = sb.tile([128, BL], F32, tag="pc", bufs=3)
                    nc.vector.tensor_scalar(pc[:], pfall[:, k * BL:(k + 1) * BL],
                                            CLIP_LO, CLIP_HI,
                                            AluOpType.max, AluOpType.min)
                    lnp = sb.tile([128, BL], F32, tag="lnp", bufs=3)
                    nc.scalar.activation(lnp[:], pc[:], AF.Ln)
                    ln1p = sb.tile([128, BL], F32, tag="ln1p", bufs=3)
                    nc.scalar.activation(ln1p[:], pc[:], AF.Ln, bias=1.0, scale=-1.0,
                                         accum_out=flacc[:, layer * 2 * NK_H + k:
                                                         layer * 2 * NK_H + k + 1])
                    d = sb.tile([128, BL], F32, tag="dln", bufs=3)
                    nc.vector.tensor_tensor(d[:], lnp[:], ln1p[:], AluOpType.subtract)
                    jf = sb.tile([128, BL], F32, tag="jf", bufs=2)
                    nc.vector.affine_mul_reduce(
                        jf[:], flacc[:, layer * 2 * NK_H + NK_H + k:
                                     layer * 2 * NK_H + NK_H + k + 1],
                        mzs, d[:], 1.0, 0.0)
                    # msum partial: mask column as stationary against raw mz
                    nc.tensor.matmul(ps_ms[:], mkh[:, k:k + 1], mzs,
                                     start=(k == 0), stop=(k == NK_H - 1))
                nc.vector.tensor_copy(ms_sb[:, layer * BL:(layer + 1) * BL], ps_ms[:])
                return mzall

            mz0all = mask_flow(mz0[:, :], pf0[:, :], mk0h, 0)
            mz1all = mask_flow(mz1[:, :], pf1[:, :], mk1h, 1)
            nc.vector.tensor_reduce(parts_t[:, 6:7], flacc[:, :], AX, AluOpType.add)

            # msum AllReduce (quad) -> mult factors broadcast to 128 partitions
            cc_ms_in = dram.tile([1, 2 * BL], F32)
            cc_ms_out = dram.tile([1, 2 * BL], F32)
            nc.sync.dma_start(cc_ms_in[:], ms_sb[:])
            nc.gpsimd.collective_compute("AllReduce", AluOpType.add,
                                         replica_groups=groups,
                                         ins=[cc_ms_in.opt()], outs=[cc_ms_out.opt()])
            ms_row = sb.tile([1, 2 * BL], F32, tag="ms_row")
            nc.sync.dma_start(ms_row[:], cc_ms_out[0:1, :])
            multbc = sb.tile([128, 2 * BL], F32, tag="multbc")
            nc.gpsimd.partition_broadcast(multbc[:], ms_row[:])
            nc.vector.tensor_scalar(multbc[:], multbc[:], 1e-6, None, AluOpType.add)
            nc.vector.reciprocal(multbc[:], multbc[:])
            nc.vector.tensor_scalar(multbc[:], multbc[:], float(H0), None, AluOpType.mult)
            mult0bc = multbc[:, 0:BL]
            mult1bc = multbc[:, BL:2 * BL]

            # ---------------- streaming weight layers ----------------
            expacc = sb.tile([128, NBLK], F32, tag="expacc")
            wacc = sb.tile([128, 2 * NBLK], F32, tag="wacc")

            def wstream(wp, G, nk, rhs_sl, with_sums, blk0, evict, wtag="w",
                        wbufs=3):
                for g in range(G):
                    pts = [ps.tile([128, BL], F32, tag="psmain", bufs=5,
                                   name=f"psm{blk0}_{g}_{m}_{id(evict)}") for m in range(4)]
                    for k in range(nk):
                        blk = sb.tile([128, 1536], F16, tag="wblk", bufs=3)
                        row = (g * nk + k) * 128
                        eng = nc.sync if (k % 2 == 0) else nc.scalar
                        eng.dma_start(blk[:], wp[row:row + 128, :])
                        mu, ls, eps = blk[:, 0:512], blk[:, 512:1024], blk[:, 1024:1536]
                        s_t = sb.tile([128, 512], F16, tag="s", bufs=3)
                        bi = blk0 + g * nk + k
                        nc.scalar.activation(
                            s_t[:], ls, AF.Exp,
                            accum_out=(expacc[:, bi:bi + 1] if with_sums else None))
                        t_t = sb.tile([128, 512], F16, tag="t", bufs=3)
                        nc.vector.tensor_tensor(t_t[:], s_t[:], eps, AluOpType.mult)
                        w_t = sb.tile([128, 512], F16, tag=wtag, bufs=wbufs)
                        nc.gpsimd.tensor_tensor(w_t[:], t_t[:], mu, AluOpType.add)
                        if with_sums:
                            psw_mm(negones_t[:], ls)            # -sum(ls)
                            j1 = sb.tile([128, 512], F16, tag="j1", bufs=2)
                            nc.vector.scalar_tensor_tensor(
                                j1[:], eps, -0.5, eps, AluOpType.mult, AluOpType.mult,
                                accum_out=wacc[:, 2 * bi:2 * bi + 1])
                            j2 = sb.tile([128, 512], F16, tag="j2", bufs=2)
                            nc.vector.scalar_tensor_tensor(
                                j2[:], mu, DECAY, mu, AluOpType.mult, AluOpType.mult,
                                accum_out=wacc[:, 2 * bi + 1:2 * bi + 2])
                        for m in range(4):
                            nc.tensor.matmul(pts[m][:], w_t[:, m * 128:(m + 1) * 128],
                                             rhs_sl(k), start=(k == 0), stop=(k == nk - 1))
                    for m in range(4):
                        evict(g * 4 + m, pts[m])

            # ---- layer 0 ----
            h0m_tiles = [None] * NK_H

            def evict0(j, pt):
                lr = sb.tile([128, BL], F32, tag="h0lr", bufs=2)
                nc.scalar.activation(lr[:], pt[:], AF.Prelu, bias=b0m[:, j:j + 1],
                                     alpha=NEG)
                hm = sb.tile([128, BL], F16, tag=f"h0m{j}", name=f"h0m{j}")
                nc.vector.scalar_tensor_tensor(hm[:], lr[:], mk0_t[:, j:j + 1],
                                               mz0all[:, j * BL:(j + 1) * BL],
                                               AluOpType.mult, AluOpType.mult)
                h0m_tiles[j] = hm

            wstream(wp0, G0, NK_IN, lambda k: xall[:, k * BL:(k + 1) * BL],
                    True, 0, evict0)

            # ---- layer 1 (partials over my H0 slice, full H1) ----
            cc_h1in = dram.tile([H1, BL], F16)
            cc_h1rs = dram.tile([H1H, BL], F16)

            def evict1(j, pt):
                hp = sb.tile([128, BL], F16, tag="h1p", bufs=2)
                nc.vector.tensor_copy(hp[:], pt[:])
                nc.gpsimd.dma_start(cc_h1in[j * 128:(j + 1) * 128, :], hp[:])

            wstream(wp1, G1, NK_H, lambda k: h0m_tiles[k][:],
                    True, G0 * NK_IN, evict1)

            nc.gpsimd.collective_compute("ReduceScatter", AluOpType.add,
                                         replica_groups=groups,
                                         ins=[cc_h1in.opt()], outs=[cc_h1rs.opt()])

            h1s = sb.tile([128, NK_H * BL], F16, tag="h1s")
            for k in range(NK_H):
                nc.sync.dma_start(h1s[:, k * BL:(k + 1) * BL],
                                  cc_h1rs[k * 128:(k + 1) * 128, :])
            h1m_tiles = []
            for k in range(NK_H):
                a = sb.tile([128, BL], F32, tag="h1a", bufs=2)
                nc.vector.tensor_tensor(a[:], h1s[:, k * BL:(k + 1) * BL], mult0bc,
                                        AluOpType.mult)
                lr = sb.tile([128, BL], F32, tag="h1lr", bufs=2)
                nc.scalar.activation(lr[:], a[:], AF.Prelu, bias=b1m[:, k:k + 1],
                                     alpha=NEG)
                hm = sb.tile([128, BL], F16, tag=f"h1m{k}", name=f"h1m{k}")
                nc.vector.scalar_tensor_tensor(hm[:], lr[:], mk1_t[:, k:k + 1],
                                               mz1all[:, k * BL:(k + 1) * BL],
                                               AluOpType.mult, AluOpType.mult)
                h1m_tiles.append(hm)

            # ---- output layer ----
            cc_pin = dram.tile([OUTP, BL], F16)
            cc_pout = dram.tile([OUTP, BL], F16)

            def evicto(j, pt):
                pp = sb.tile([128, BL], F16, tag="pp", bufs=2)
                nc.scalar.activation(pp[:], pt[:], AF.Copy)
                nc.gpsimd.dma_start(cc_pin[j * 128:(j + 1) * 128, :], pp[:])
                if j == 7:
                    nc.gpsimd.collective_compute(
                        "AllReduce", AluOpType.add, replica_groups=groups,
                        ins=[cc_pin.opt()], outs=[cc_pout.opt()])

            wstream(wpo, GO, NK_H, lambda k: h1m_tiles[k][:],
                    False, 0, evicto, wtag="wo", wbufs=6)

            par = sb.tile([128, (OUTP // 128) * BL], F16, tag="xall", name="par")
            for m in range(OUTP // 128):
                nc.sync.dma_start(par[:, m * BL:(m + 1) * BL],
                                  cc_pout[m * 128:(m + 1) * 128, :])
            predT_tiles = []
            for m in range(OUTP // 128):
                pm = sb.tile([128, BL], F32, tag="pTa", bufs=2)
                nc.vector.tensor_tensor(pm[:], par[:, m * BL:(m + 1) * BL], mult1bc,
                                        AluOpType.mult)
                pt2 = sb.tile([128, BL], F32, tag=f"pT{m}", name=f"pT{m}")
                nc.vector.tensor_scalar(pt2[:], pm[:], bom[:, m:m + 1], None,
                                        AluOpType.add)
                predT_tiles.append(pt2)

            # transpose predT [1024, BL] -> pred [BL, 1024]
            idn = sb.tile([128, 128], F32, tag="idn")
            nc.vector.tensor_scalar(idn[:], arbc[:, 0:128], p128_t[:, 0:1], None,
                                    AluOpType.is_equal)
            # softmax / CE partials per batch tile (2 rotating pred buffers)
            ceacc = sb.tile([128, NBT], F32, tag="ceacc")
            for bt in range(NBT):
                psb = sb.tile([128, OUTP], F32, tag="psb", bufs=2, name=f"psb{bt}")
                for m in range(OUTP // 128):
                    tr = ps.tile([128, 128], F32, tag="pstr", bufs=1,
                                 name=f"tr{m}_{bt}")
                    nc.tensor.transpose(tr[:], predT_tiles[m][:, bt * 128:(bt + 1) * 128],
                                        idn[:])
                    nc.scalar.activation(psb[:, m * 128:(m + 1) * 128], tr[:],
                                         AF.Copy)
                nc.sync.dma_start(pred_out[bt * 128:(bt + 1) * 128, :],
                                  psb[:, 0:OUT])
                negmax = sb.tile([128, 1], F32, tag="negmax", bufs=2)
                nc.vector.tensor_reduce(negmax[:], psb[:], AX, AluOpType.max,
                                        negate=True)
                je = sb.tile([128, OUTP], F16, tag="je", bufs=1)
                sumexp = sb.tile([128, 1], F32, tag="sumexp", bufs=2)
                nc.scalar.activation(je[:], psb[:], AF.Exp, bias=negmax[:, 0:1],
                                     accum_out=sumexp[:])
                lse = sb.tile([128, 1], F32, tag="lse", bufs=2)
                nc.scalar.activation(lse[:], sumexp[:], AF.Ln)
                iseq = sb.tile([128, OUTP], F16, tag="iseq", bufs=1)
                nc.vector.tensor_scalar(iseq[:], arbc[:], y_t[:, bt:bt + 1], None,
                                        AluOpType.is_equal)
                jp = sb.tile([128, OUTP], F32, tag="jp", bufs=1)
                py = sb.tile([128, 1], F32, tag="py", bufs=2)
                nc.vector.affine_mul_reduce(jp[:], py[:], iseq[:], psb[:],
                                            1.0, 0.0)
                tmp = sb.tile([128, 1], F32, tag="cetmp", bufs=2)
                nc.vector.tensor_tensor(tmp[:], py[:], negmax[:], AluOpType.add)
                nc.vector.tensor_tensor(ceacc[:, bt:bt + 1], tmp[:], lse[:],
                                        AluOpType.subtract)
            nc.vector.tensor_reduce(parts_t[:, 8:9], ceacc[:, :], AX, AluOpType.add)

            # final partial columns
            nc.vector.tensor_reduce(parts_t[0:1, 0:1], ps_w[:], AX, AluOpType.add)
            nc.vector.tensor_reduce(parts_t[:, 1:2], expacc[:], AX, AluOpType.add)
            nc.vector.tensor_reduce(parts_t[:, 9:10], wacc[:], AX, AluOpType.add)
            nc.sync.dma_start(partials[:, :], parts_t[:])

    nc.compile()
    return nc


def _prep(inputs):
    f16, f32 = np.float16, np.float32

    def pack_layer(mu, ls, eps, G, nk):
        # arrays [I_local, O_local] f16 -> [G*nk*128, 1536] block-contiguous
        def blocks(a):
            return a.reshape(nk, 128, G, 512).transpose(2, 0, 1, 3)
        out = np.concatenate([blocks(mu), blocks(ls), blocks(eps)], axis=3)
        return np.ascontiguousarray(out).reshape(G * nk * 128, 1536)

    packs = {}
    for r in range(TP):
        sl0 = slice(r * H0H, (r + 1) * H0H)
        packs[("wp0", r)] = pack_layer(inputs["w_mu0"].T[:, sl0].astype(f16),
                                       inputs["w_ls0"].T[:, sl0].astype(f16),
                                       inputs["eps_w0"].T[:, sl0].astype(f16),
                                       G0, NK_IN)
        packs[("wp1", r)] = pack_layer(inputs["w_mu1"].T[sl0, :].astype(f16),
                                       inputs["w_ls1"].T[sl0, :].astype(f16),
                                       inputs["eps_w1"].T[sl0, :].astype(f16),
                                       G1, NK_H)
        slo = slice(r * H1H, (r + 1) * H1H)

        def padded(a):
            out = np.zeros((H1H, OUTP), f16)
            out[:, :OUT] = a.T[slo, :].astype(f16)
            return out
        packs[("wpo", r)] = pack_layer(padded(inputs["w_muo"]),
                                       padded(inputs["w_lso"]),
                                       padded(inputs["eps_wo"]), GO, NK_H)

    def vec128(v, ncol):
        return np.ascontiguousarray(v.reshape(ncol, 128).T.astype(f32))

    NC = H0H // 128
    bvs = {}
    for r in range(TP):
        for nm in ("0", "1"):
            sl = slice(r * H0H, (r + 1) * H0H)
            bvs[(f"bv{nm}", r)] = np.concatenate(
                [vec128(inputs["b_mu" + nm][sl], NC),
                 vec128(inputs["b_ls" + nm][sl], NC),
                 vec128(inputs["eps_b" + nm][sl], NC)], axis=1)
        bmo = np.full(OUTP, -1e30, f32); bmo[:OUT] = inputs["b_muo"]
        blo = np.zeros(OUTP, f32); blo[:OUT] = inputs["b_lso"]
        beo = np.zeros(OUTP, f32); beo[:OUT] = inputs["eps_bo"]
        bvs[("bvo", r)] = np.concatenate(
            [vec128(bmo, 8), vec128(blo, 8), vec128(beo, 8)], axis=1)
        bvs[("mk0", r)] = vec128(inputs["mask_mu0"][sl].astype(f32), NC)
        bvs[("mk1", r)] = vec128(inputs["mask_mu1"][sl].astype(f32), NC)

    zw2 = np.ascontiguousarray(
        (inputs["z_w"].astype(f32) / TP).reshape(NK_IN, 128).T.astype(f16))
    ar1 = np.arange(OUTP, dtype=f32)[None, :]
    p128a = np.arange(128, dtype=f32)[:, None]

    xTf = inputs["x"].T.astype(f16)
    pf0T = inputs["pfz0"].T.astype(f16)
    pf1T = inputs["pfz1"].T.astype(f16)
    mz0T = inputs["mz0"].T.astype(f16)
    mz1T = inputs["mz1"].T.astype(f16)

    in_maps = []
    for c in range(8):
        p, r = c // TP, c % TP
        bsl = slice(p * BL, (p + 1) * BL)
        hsl = slice(r * H0H, (r + 1) * H0H)
        m = {
            "wp0": packs[("wp0", r)], "wp1": packs[("wp1", r)],
            "wpo": packs[("wpo", r)],
            "xT": np.ascontiguousarray(xTf[:, bsl]),
            "pf0": np.ascontiguousarray(pf0T[hsl, bsl]),
            "pf1": np.ascontiguousarray(pf1T[hsl, bsl]),
            "mz0": np.ascontiguousarray(mz0T[hsl, bsl]),
            "mz1": np.ascontiguousarray(mz1T[hsl, bsl]),
            "bv0": bvs[("bv0", r)], "bv1": bvs[("bv1", r)],
            "bvo": bvs[("bvo", r)],
            "mk0": bvs[("mk0", r)], "mk1": bvs[("mk1", r)],
            "zw2": zw2,
            "y2": np.ascontiguousarray(
                inputs["y"][bsl].astype(f32).reshape(NBT, 128).T),
            "ar1": ar1, "p128": p128a,
        }
        in_maps.append(m)
    return in_maps


def kernel(**inputs):
    if "nc" not in _CACHE:
        _CACHE["nc"] = _build()
    nc = _CACHE["nc"]
    in_maps = _prep(inputs)
    res = run_bass_kernel_spmd(nc, in_maps, core_ids=list(range(8)),
                               **_CACHE.get("run_kwargs", {}))
    _CACHE["last_res"] = res
    outs = res.results

    pred = np.concatenate([outs[TP * p]["pred_out"] for p in range(DP)], axis=0)

    s = [outs[c]["partials"].astype(np.float64).sum(axis=0) for c in range(8)]
    g0 = range(TP)                       # one DP group's ranks cover all shards
    pairs_dedup = [TP * p for p in range(DP)]
    psw = sum(s[c][0] for c in g0)       # -Σls (w) + xz (zw pre-scaled by 1/TP)
    expsum = sum(s[c][1] for c in g0)    # Σ exp(ls), both w matrices
    bls = sum(s[c][2] + s[c][3] for c in g0)
    beb2 = sum(s[c][4] + s[c][5] for c in g0)
    flow = sum(s[c][6] for c in range(8))
    ce_raw = sum(s[c][8] for c in pairs_dedup)
    wsq = sum(s[c][9] for c in g0)       # -0.5Σeps² + 0.01Σmu²

    n_w = H0 * IN + H1 * H0
    n_b = H0 + H1
    L = (psw + wsq + flow
         + n_w * C0
         + n_b * C0 - bls - 0.5 * beb2
         + 60000.0 * (-ce_raw / B)
         + DECAY * expsum)
    loss = np.float32(np.float32(L) ** 2)
    return pred, loss


# revision 27
# speedup vs baseline: 87.9553x; 1.0836x over previous
"""Bass/Trainium2 kernel for nn_BaysianMLPMaskedDropout (8 NeuronCores).

Sharding: 2 data-parallel groups (batch 1024 -> 2 x 512) x 4-way tensor
parallel (the 4096 hidden axis of each BayesLinear split in quarters).
Activations are kept feature-major ("transposed", [feature, batch]) on chip so
every matmul contracts along the partition axis with no transposes between
layers.  A per-quad ReduceScatter after layer 1 and AllReduce after the output
layer run on the collective engine.  The scalar flow-loss pieces are reduced
on-device into per-core partial vectors and combined on the host.

log(pw) = log(cdf(w+eps)-cdf(w-eps)) is evaluated with the exact expansion
log(2*EPS/sqrt(2pi)) - w_ls - eps^2/2 (error < 1e-7, far below the f32 noise
of the reference's catastrophically-cancelling cdf difference).
"""

import numpy as np

import concourse.bacc as bacc
import concourse.tile as tile
from concourse import hw_specs as _hw_specs

# All ACT functions this kernel uses (Exp, Ln, Lrelu, Copy, Square) co-reside
# in the natural_log_exp_and_others table set; restrict the set chooser to it
# so the scheduler emits one table load instead of ~26 (2.7us each).
_orig_gat = _hw_specs.get_activation_tables

def _gat_one_set(arch):
    tabs = _orig_gat(arch)
    pref = "natural_log_exp_and_others"
    if pref in tabs:
        tabs = {k: (v if k == pref else set()) for k, v in tabs.items()}
    return tabs

bacc.get_activation_tables = _gat_one_set
from concourse import mybir
from concourse.alu_op_type import AluOpType
from concourse.bass_utils import run_bass_kernel_spmd

F16 = mybir.dt.float16
F32 = mybir.dt.float32
AF = mybir.ActivationFunctionType
AX = mybir.AxisListType.X

B, IN, H0, H1, OUT = 1024, 2048, 4096, 4096, 1000
OUTP = 1024                      # classes padded to 8*128 for uniform tiles
TP, DP = 4, 2
BL = B // DP                     # batch rows per DP group
NBT = BL // 128                  # batch tiles (softmax / transpose)
H0H, H1H = H0 // TP, H1 // TP    # per-rank hidden slice
NK_IN = IN // 128                # contraction chunks, layer 0
NK_H = H0H // 128                # contraction chunks, layers 1/o
G0, G1, GO = H0H // 512, H1 // 512, OUTP // 512
NBLK = G0 * NK_IN + G1 * NK_H    # weight-stream blocks with flow sums
EPS_CDF = 1e-5
C0 = float(np.log(2 * EPS_CDF / np.sqrt(2 * np.pi)))
NEG = 0.01
DECAY = 0.01
CLIP_LO = float(np.float32(1e-6))
CLIP_HI = float(np.float32(1.0 - 1e-6))

_CACHE = {}


def _build():
    nc = bacc.Bacc("TRN2", debug=False, num_devices=8, enable_asserts=False)

    def din(name, shape, dt=F16):
        return nc.dram_tensor(name, shape, dt, kind="ExternalInput").ap()

    wp0 = din("wp0", [G0 * NK_IN * 128, 1536])
    wp1 = din("wp1", [G1 * NK_H * 128, 1536])
    wpo = din("wpo", [GO * NK_H * 128, 1536])
    xT = din("xT", [IN, BL])
    pf0 = din("pf0", [H0H, BL])
    pf1 = din("pf1", [H1H, BL])
    mz0 = din("mz0", [H0H, BL])
    mz1 = din("mz1", [H1H, BL])
    NC = H0H // 128              # bias columns per layer slice
    bv0 = din("bv0", [128, 3 * NC], F32)   # [mu | ls | eps]
    bv1 = din("bv1", [128, 3 * NC], F32)
    bvo = din("bvo", [128, 24], F32)       # padded to 1024 classes
    mk0 = din("mk0", [128, NC], F32)
    mk1 = din("mk1", [128, NC], F32)
    zw2 = din("zw2", [128, NK_IN])         # f16, pre-scaled by 1/TP
    y2 = din("y2", [128, NBT], F32)
    ar1 = din("ar1", [1, OUTP], F32)
    p128 = din("p128", [128, 1], F32)

    pred_out = nc.dram_tensor("pred_out", [BL, OUT], F32, kind="ExternalOutput").ap()
    partials = nc.dram_tensor("partials", [128, 16], F32, kind="ExternalOutput").ap()

    groups = [list(range(i, i + TP)) for i in range(0, 8, TP)]

    with tile.TileContext(nc) as tc:
        with (
            tc.tile_pool(name="sb", bufs=1) as sb,
            tc.tile_pool(name="ps", bufs=1, space="PSUM") as ps,
            tc.tile_pool(name="dram", bufs=1, space="DRAM") as dram,
        ):
            # ---------------- small constant loads ----------------
            def load(name, src, shape, dt=F32):
                t = sb.tile(shape, dt, tag=name)
                nc.sync.dma_start(t[:], src)
                return t

            bv0_t = load("bv0", bv0[:, :], [128, 3 * NC])
            bv1_t = load("bv1", bv1[:, :], [128, 3 * NC])
            bvo_t = load("bvo", bvo[:, :], [128, 24])
            mk0_t = load("mk0", mk0[:, :], [128, NC])
            mk1_t = load("mk1", mk1[:, :], [128, NC])
            zw_t = load("zw", zw2[:, :], [128, NK_IN], F16)
            y_t = load("y", y2[:, :], [128, NBT])
            p128_t = load("p128", p128[:, :], [128, 1])
            ar_row = sb.tile([1, OUTP], F32, tag="ar_row")
            nc.sync.dma_start(ar_row[:], ar1[0:1, :])
            arbc = sb.tile([128, OUTP], F32, tag="arbc")
            nc.gpsimd.partition_broadcast(arbc[:], ar_row[:])
            # f16 copies of the mask columns (used as matmul lhsT for msum)
            mk0h = sb.tile([128, NC], F16, tag="mk0h")
            nc.vector.tensor_copy(mk0h[:], mk0_t[:])
            mk1h = sb.tile([128, NC], F16, tag="mk1h")
            nc.vector.tensor_copy(mk1h[:], mk1_t[:])

            negones_t = sb.tile([128, 1], F16, tag="negones")
            nc.vector.memset(negones_t[:], -1.0)

            parts_t = sb.tile([128, 16], F32, tag="parts")
            nc.vector.memset(parts_t[:], 0.0)

            # bias materialization: b = b_mu + exp(b_ls) * b_eps
            def bias_mat(bv, ncol, tag):
                s = sb.tile([128, ncol], F32, tag=tag + "s")
                nc.scalar.activation(s[:], bv[:, ncol:2 * ncol], AF.Exp)
                t = sb.tile([128, ncol], F32, tag=tag + "t")
                nc.vector.tensor_tensor(t[:], s[:], bv[:, 2 * ncol:3 * ncol], AluOpType.mult)
                b = sb.tile([128, ncol], F32, tag=tag + "b")
                nc.vector.tensor_tensor(b[:], t[:], bv[:, 0:ncol], AluOpType.add)
                return b

            b0m = bias_mat(bv0_t, NC, "b0")
            b1m = bias_mat(bv1_t, NC, "b1")
            bom = bias_mat(bvo_t, 8, "bo")

            # b-vector flow partials: sum(b_ls), sum(b_eps^2) for layers 0,1
            nc.vector.tensor_reduce(parts_t[:, 2:3], bv0_t[:, NC:2 * NC], AX, AluOpType.add)
            nc.vector.tensor_reduce(parts_t[:, 3:4], bv1_t[:, NC:2 * NC], AX, AluOpType.add)
            jb = sb.tile([128, NC], F32, tag="jb", bufs=2)
            nc.vector.affine_mul_reduce(jb[:], parts_t[:, 4:5], bv0_t[:, 2 * NC:3 * NC],
                                        bv0_t[:, 2 * NC:3 * NC], 1.0, 0.0)
            jb2 = sb.tile([128, NC], F32, tag="jb", bufs=2)
            nc.vector.affine_mul_reduce(jb2[:], parts_t[:, 5:6], bv1_t[:, 2 * NC:3 * NC],
                                        bv1_t[:, 2 * NC:3 * NC], 1.0, 0.0)

            # ---------------- x load ----------------
            xall = sb.tile([128, NK_IN * BL], F16, tag="xall")
            for k in range(NK_IN):
                nc.sync.dma_start(xall[:, k * BL:(k + 1) * BL],
                                  xT[k * 128:(k + 1) * 128, :])

            # combined scalar PSUM row: -sum(ls) + 0.01*sum(mu^2 via DVE? no –
            # ls via neg-ones matmuls) + xz/TP via zw-column matmuls
            ps_w = ps.tile([1, 512], F32, tag="psw")
            n_psw = NBLK + NK_IN   # total accumulating matmuls into ps_w
            psw_i = [0]

            def psw_mm(lhsT, rhs):
                nc.tensor.matmul(ps_w[:], lhsT, rhs, start=(psw_i[0] == 0),
                                 stop=(psw_i[0] == n_psw - 1), skip_group_check=True)
                psw_i[0] += 1

            # xz/TP partial: zw columns (pre-scaled 1/TP) against x chunks
            for k in range(NK_IN):
                psw_mm(zw_t[:, k:k + 1], xall[:, k * BL:(k + 1) * BL])

            # ---------------- masks + pfz flow (both layers) ----------------
            flacc = sb.tile([128, 4 * NK_H], F32, tag="flacc")
            ps_ms = ps.tile([1, BL], F32, tag="psms")
            ms_sb = sb.tile([1, 2 * BL], F32, tag="ms_sb")

            def mask_flow(mzap, pfap, mkh, layer):
                mzall = sb.tile([128, NK_H * BL], F16, tag=f"mzall{layer}",
                                name=f"mzall{layer}")
                pfall = sb.tile([128, NK_H * BL], F16, tag="pfall",
                                name=f"pfall{layer}")
                for k in range(NK_H):
                    nc.sync.dma_start(mzall[:, k * BL:(k + 1) * BL],
                                      mzap[k * 128:(k + 1) * 128, :])
                    nc.sync.dma_start(pfall[:, k * BL:(k + 1) * BL],
                                      pfap[k * 128:(k + 1) * 128, :])
                for k in range(NK_H):
                    mzs = mzall[:, k * BL:(k + 1) * BL]
                    pc = sb.tile([128, BL], F32, tag="pc", bufs=3)
                    nc.vector.tensor_scalar(pc[:], pfall[:, k * BL:(k + 1) * BL],
                                            CLIP_LO, CLIP_HI,
                                            AluOpType.max, AluOpType.min)
                    lnp = sb.tile([128, BL], F32, tag="lnp", bufs=3)
                    nc.scalar.activation(lnp[:], pc[:], AF.Ln)
                    ln1p = sb.tile([128, BL], F32, tag="ln1p", bufs=3)
                    nc.scalar.activation(ln1p[:], pc[:], AF.Ln, bias=1.0, scale=-1.0,
                                         accum_out=flacc[:, layer * 2 * NK_H + k:
                                                         layer * 2 * NK_H + k + 1])
                    d = sb.tile([128, BL], F32, tag="dln", bufs=3)
                    nc.vector.tensor_tensor(d[:], lnp[:], ln1p[:], AluOpType.subtract)
                    jf = sb.tile([128, BL], F32, tag="jf", bufs=2)
                    nc.vector.affine_mul_reduce(
                        jf[:], flacc[:, layer * 2 * NK_H + NK_H + k:
                                     layer * 2 * NK_H + NK_H + k + 1],
                        mzs, d[:], 1.0, 0.0)
                    # msum partial: mask column as stationary against raw mz
                    nc.tensor.matmul(ps_ms[:], mkh[:, k:k + 1], mzs,
                                     start=(k == 0), stop=(k == NK_H - 1))
                nc.vector.tensor_copy(ms_sb[:, layer * BL:(layer + 1) * BL], ps_ms[:])
                return mzall

            mz0all = mask_flow(mz0[:, :], pf0[:, :], mk0h, 0)
            mz1all = mask_flow(mz1[:, :], pf1[:, :], mk1h, 1)
            nc.vector.tensor_reduce(parts_t[:, 6:7], flacc[:, :], AX, AluOpType.add)

            # msum AllReduce (quad) -> mult factors broadcast to 128 partitions
            cc_ms_in = dram.tile([1, 2 * BL], F32)
            cc_ms_out = dram.tile([1, 2 * BL], F32)
            nc.sync.dma_start(cc_ms_in[:], ms_sb[:])
            nc.gpsimd.collective_compute("AllReduce", AluOpType.add,
                                         replica_groups=groups,
                                         ins=[cc_ms_in.opt()], outs=[cc_ms_out.opt()])
            ms_row = sb.tile([1, 2 * BL], F32, tag="ms_row")
            nc.sync.dma_start(ms_row[:], cc_ms_out[0:1, :])
            multbc = sb.tile([128, 2 * BL], F32, tag="multbc")
            nc.gpsimd.partition_broadcast(multbc[:], ms_row[:])
            nc.vector.tensor_scalar(multbc[:], multbc[:], 1e-6, None, AluOpType.add)
            nc.vector.reciprocal(multbc[:], multbc[:])
            nc.vector.tensor_scalar(multbc[:], multbc[:], float(H0), None, AluOpType.mult)
            mult0bc = multbc[:, 0:BL]
            mult1bc = multbc[:, BL:2 * BL]

            # ---------------- streaming weight layers ----------------
            expacc = sb.tile([128, NBLK], F32, tag="expacc")
            wacc = sb.tile([128, 2 * NBLK], F32, tag="wacc")

            def wstream(wp, G, nk, rhs_sl, with_sums, blk0, evict, wtag="w",
                        wbufs=4):
                for g in range(G):
                    pts = [ps.tile([128, BL], F32, tag="psmain", bufs=5,
                                   name=f"psm{blk0}_{g}_{m}_{id(evict)}") for m in range(4)]
                    for k in range(nk):
                        blk = sb.tile([128, 1536], F16, tag="wblk", bufs=5)
                        row = (g * nk + k) * 128
                        eng = nc.sync if (k % 2 == 0) else nc.scalar
                        eng.dma_start(blk[:], wp[row:row + 128, :])
                        mu, ls, eps = blk[:, 0:512], blk[:, 512:1024], blk[:, 1024:1536]
                        s_t = sb.tile([128, 512], F16, tag="s", bufs=4)
                        bi = blk0 + g * nk + k
                        nc.scalar.activation(
                            s_t[:], ls, AF.Exp,
                            accum_out=(expacc[:, bi:bi + 1] if with_sums else None))
                        t_t = sb.tile([128, 512], F16, tag="t", bufs=4)
                        nc.vector.tensor_tensor(t_t[:], s_t[:], eps, AluOpType.mult)
                        w_t = sb.tile([128, 512], F16, tag=wtag, bufs=wbufs)
                        nc.gpsimd.tensor_tensor(w_t[:], t_t[:], mu, AluOpType.add)
                        if with_sums:
                            psw_mm(negones_t[:], ls)            # -sum(ls)
                            j1 = sb.tile([128, 512], F16, tag="j1", bufs=2)
                            nc.vector.scalar_tensor_tensor(
                                j1[:], eps, -0.5, eps, AluOpType.mult, AluOpType.mult,
                                accum_out=wacc[:, 2 * bi:2 * bi + 1])
                            j2 = sb.tile([128, 512], F16, tag="j2", bufs=2)
                            nc.vector.scalar_tensor_tensor(
                                j2[:], mu, DECAY, mu, AluOpType.mult, AluOpType.mult,
                                accum_out=wacc[:, 2 * bi + 1:2 * bi + 2])
                        for m in range(4):
                            nc.tensor.matmul(pts[m][:], w_t[:, m * 128:(m + 1) * 128],
                                             rhs_sl(k), start=(k == 0), stop=(k == nk - 1))
                    for m in range(4):
                        evict(g * 4 + m, pts[m])

            # ---- layer 0 ----
            h0m_tiles = [None] * NK_H

            def evict0(j, pt):
                lr = sb.tile([128, BL], F32, tag="h0lr", bufs=2)
                nc.scalar.activation(lr[:], pt[:], AF.Prelu, bias=b0m[:, j:j + 1],
                                     alpha=NEG)
                hm = sb.tile([128, BL], F16, tag=f"h0m{j}", name=f"h0m{j}")
                nc.vector.scalar_tensor_tensor(hm[:], lr[:], mk0_t[:, j:j + 1],
                                               mz0all[:, j * BL:(j + 1) * BL],
                                               AluOpType.mult, AluOpType.mult)
                h0m_tiles[j] = hm

            wstream(wp0, G0, NK_IN, lambda k: xall[:, k * BL:(k + 1) * BL],
                    True, 0, evict0)

            # ---- layer 1 (partials over my H0 slice, full H1) ----
            cc_h1in = dram.tile([H1, BL], F16)
            cc_h1rs = dram.tile([H1H, BL], F16)

            def evict1(j, pt):
                hp = sb.tile([128, BL], F16, tag="h1p", bufs=4)
                nc.vector.tensor_copy(hp[:], pt[:])
                nc.gpsimd.dma_start(cc_h1in[j * 128:(j + 1) * 128, :], hp[:])

            wstream(wp1, G1, NK_H, lambda k: h0m_tiles[k][:],
                    True, G0 * NK_IN, evict1)

            nc.gpsimd.collective_compute("ReduceScatter", AluOpType.add,
                                         replica_groups=groups,
                                         ins=[cc_h1in.opt()], outs=[cc_h1rs.opt()])

            h1s = sb.tile([128, NK_H * BL], F16, tag="h1s")
            for k in range(NK_H):
                nc.sync.dma_start(h1s[:, k * BL:(k + 1) * BL],
                                  cc_h1rs[k * 128:(k + 1) * 128, :])
            h1m_tiles = []
            for k in range(NK_H):
                a = sb.tile([128, BL], F32, tag="h1a", bufs=2)
                nc.vector.tensor_tensor(a[:], h1s[:, k * BL:(k + 1) * BL], mult0bc,
                                        AluOpType.mult)
                lr = sb.tile([128, BL], F32, tag="h1lr", bufs=2)
                nc.scalar.activation(lr[:], a[:], AF.Prelu, bias=b1m[:, k:k + 1],
                                     alpha=NEG)
                hm = sb.tile([128, BL], F16, tag=f"h1m{k}", name=f"h1m{k}")
                nc.vector.scalar_tensor_tensor(hm[:], lr[:], mk1_t[:, k:k + 1],
                                               mz1all[:, k * BL:(k + 1) * BL],
                                               AluOpType.mult, AluOpType.mult)
                h1m_tiles.append(hm)

            # ---- output layer ----
            cc_pin = dram.tile([OUTP, BL], F16)
            cc_pout = dram.tile([OUTP, BL], F16)

            def evicto(j, pt):
                pp = sb.tile([128, BL], F16, tag="pp", bufs=2)
                nc.scalar.activation(pp[:], pt[:], AF.Copy)
                nc.gpsimd.dma_start(cc_pin[j * 128:(j + 1) * 128, :], pp[:])
                if j == 7:
                    nc.gpsimd.collective_compute(
                        "AllReduce", AluOpType.add, replica_groups=groups,
                        ins=[cc_pin.opt()], outs=[cc_pout.opt()])

            wstream(wpo, GO, NK_H, lambda k: h1m_tiles[k][:],
                    False, 0, evicto, wtag="wo", wbufs=6)

            par = sb.tile([128, (OUTP // 128) * BL], F16, tag="xall", name="par")
            for m in range(OUTP // 128):
                nc.sync.dma_start(par[:, m * BL:(m + 1) * BL],
                                  cc_pout[m * 128:(m + 1) * 128, :])
            predT_tiles = []
            for m in range(OUTP // 128):
                pm = sb.tile([128, BL], F32, tag="pTa", bufs=2)
                nc.vector.tensor_tensor(pm[:], par[:, m * BL:(m + 1) * BL], mult1bc,
                                        AluOpType.mult)
                pt2 = sb.tile([128, BL], F32, tag=f"pT{m}", name=f"pT{m}")
                nc.vector.tensor_scalar(pt2[:], pm[:], bom[:, m:m + 1], None,
                                        AluOpType.add)
                predT_tiles.append(pt2)

            # transpose predT [1024, BL] -> pred [BL, 1024]
            idn = sb.tile([128, 128], F32, tag="idn")
            nc.vector.tensor_scalar(idn[:], arbc[:, 0:128], p128_t[:, 0:1], None,
                                    AluOpType.is_equal)
            # softmax / CE partials per batch tile (2 rotating pred buffers)
            ceacc = sb.tile([128, NBT], F32, tag="ceacc")
            for bt in range(NBT):
                psb = sb.tile([128, OUTP], F32, tag="psb", bufs=2, name=f"psb{bt}")
                for m in range(OUTP // 128):
                    tr = ps.tile([128, 128], F32, tag="pstr", bufs=1,
                                 name=f"tr{m}_{bt}")
                    nc.tensor.transpose(tr[:], predT_tiles[m][:, bt * 128:(bt + 1) * 128],
                                        idn[:])
                    nc.scalar.activation(psb[:, m * 128:(m + 1) * 128], tr[:],
                                         AF.Copy)
                nc.sync.dma_start(pred_out[bt * 128:(bt + 1) * 128, :],
                                  psb[:, 0:OUT])
                negmax = sb.tile([128, 1], F32, tag="negmax", bufs=2)
                nc.vector.tensor_reduce(negmax[:], psb[:], AX, AluOpType.max,
                                        negate=True)
                je = sb.tile([128, OUTP], F16, tag="je", bufs=1)
                sumexp = sb.tile([128, 1], F32, tag="sumexp", bufs=2)
                nc.scalar.activation(je[:], psb[:], AF.Exp, bias=negmax[:, 0:1],
                                     accum_out=sumexp[:])
                lse = sb.tile([128, 1], F32, tag="lse", bufs=2)
                nc.scalar.activation(lse[:], sumexp[:], AF.Ln)
                iseq = sb.tile([128, OUTP], F16, tag="iseq", bufs=1)
                nc.vector.tensor_scalar(iseq[:], arbc[:], y_t[:, bt:bt + 1], None,
                                        AluOpType.is_equal)
                jp = sb.tile([128, OUTP], F32, tag="jp", bufs=1)
                py = sb.tile([128, 1], F32, tag="py", bufs=2)
                nc.vector.affine_mul_reduce(jp[:], py[:], iseq[:], psb[:],
                                            1.0, 0.0)
                tmp = sb.tile([128, 1], F32, tag="cetmp", bufs=2)
                nc.vector.tensor_tensor(tmp[:], py[:], negmax[:], AluOpType.add)
                nc.vector.tensor_tensor(ceacc[:, bt:bt + 1], tmp[:], lse[:],
                                        AluOpType.subtract)
            nc.vector.tensor_reduce(parts_t[:, 8:9], ceacc[:, :], AX, AluOpType.add)

            # final partial columns
            nc.vector.tensor_reduce(parts_t[0:1, 0:1], ps_w[:], AX, AluOpType.add)
            nc.vector.tensor_reduce(parts_t[:, 1:2], expacc[:], AX, AluOpType.add)
            nc.vector.tensor_reduce(parts_t[:, 9:10], wacc[:], AX, AluOpType.add)
            nc.sync.dma_start(partials[:, :], parts_t[:])

    nc.compile()
    return nc


def _prep(inputs):
    f16, f32 = np.float16, np.float32

    def pack_layer(mu, ls, eps, G, nk):
        # arrays [I_local, O_local] f16 -> [G*nk*128, 1536] block-contiguous
        def blocks(a):
            return a.reshape(nk, 128, G, 512).transpose(2, 0, 1, 3)
        out = np.concatenate([blocks(mu), blocks(ls), blocks(eps)], axis=3)
        return np.ascontiguousarray(out).reshape(G * nk * 128, 1536)

    packs = {}
    for r in range(TP):
        sl0 = slice(r * H0H, (r + 1) * H0H)
        packs[("wp0", r)] = pack_layer(inputs["w_mu0"].T[:, sl0].astype(f16),
                                       inputs["w_ls0"].T[:, sl0].astype(f16),
                                       inputs["eps_w0"].T[:, sl0].astype(f16),
                                       G0, NK_IN)
        packs[("wp1", r)] = pack_layer(inputs["w_mu1"].T[sl0, :].astype(f16),
                                       inputs["w_ls1"].T[sl0, :].astype(f16),
                                       inputs["eps_w1"].T[sl0, :].astype(f16),
                                       G1, NK_H)
        slo = slice(r * H1H, (r + 1) * H1H)

        def padded(a):
            out = np.zeros((H1H, OUTP), f16)
            out[:, :OUT] = a.T[slo, :].astype(f16)
            return out
        packs[("wpo", r)] = pack_layer(padded(inputs["w_muo"]),
                                       padded(inputs["w_lso"]),
                                       padded(inputs["eps_wo"]), GO, NK_H)

    def vec128(v, ncol):
        return np.ascontiguousarray(v.reshape(ncol, 128).T.astype(f32))

    NC = H0H // 128
    bvs = {}
    for r in range(TP):
        for nm in ("0", "1"):
            sl = slice(r * H0H, (r + 1) * H0H)
            bvs[(f"bv{nm}", r)] = np.concatenate(
                [vec128(inputs["b_mu" + nm][sl], NC),
                 vec128(inputs["b_ls" + nm][sl], NC),
                 vec128(inputs["eps_b" + nm][sl], NC)], axis=1)
        bmo = np.full(OUTP, -1e30, f32); bmo[:OUT] = inputs["b_muo"]
        blo = np.zeros(OUTP, f32); blo[:OUT] = inputs["b_lso"]
        beo = np.zeros(OUTP, f32); beo[:OUT] = inputs["eps_bo"]
        bvs[("bvo", r)] = np.concatenate(
            [vec128(bmo, 8), vec128(blo, 8), vec128(beo, 8)], axis=1)
        bvs[("mk0", r)] = vec128(inputs["mask_mu0"][sl].astype(f32), NC)
        bvs[("mk1", r)] = vec128(inputs["mask_mu1"][sl].astype(f32), NC)

    zw2 = np.ascontiguousarray(
        (inputs["z_w"].astype(f32) / TP).reshape(NK_IN, 128).T.astype(f16))
    ar1 = np.arange(OUTP, dtype=f32)[None, :]
    p128a = np.arange(128, dtype=f32)[:, None]

    xTf = inputs["x"].T.astype(f16)
    pf0T = inputs["pfz0"].T.astype(f16)
    pf1T = inputs["pfz1"].T.astype(f16)
    mz0T = inputs["mz0"].T.astype(f16)
    mz1T = inputs["mz1"].T.astype(f16)

    in_maps = []
    for c in range(8):
        p, r = c // TP, c % TP
        bsl = slice(p * BL, (p + 1) * BL)
        hsl = slice(r * H0H, (r + 1) * H0H)
        m = {
            "wp0": packs[("wp0", r)], "wp1": packs[("wp1", r)],
            "wpo": packs[("wpo", r)],
            "xT": np.ascontiguousarray(xTf[:, bsl]),
            "pf0": np.ascontiguousarray(pf0T[hsl, bsl]),
            "pf1": np.ascontiguousarray(pf1T[hsl, bsl]),
            "mz0": np.ascontiguousarray(mz0T[hsl, bsl]),
            "mz1": np.ascontiguousarray(mz1T[hsl, bsl]),
            "bv0": bvs[("bv0", r)], "bv1": bvs[("bv1", r)],
            "bvo": bvs[("bvo", r)],
            "mk0": bvs[("mk0", r)], "mk1": bvs[("mk1", r)],
            "zw2": zw2,
            "y2": np.ascontiguousarray(
                inputs["y"][bsl].astype(f32).reshape(NBT, 128).T),
            "ar1": ar1, "p128": p128a,
        }
        in_maps.append(m)
    return in_maps


def kernel(**inputs):
    if "nc" not in _CACHE:
        _CACHE["nc"] = _build()
    nc = _CACHE["nc"]
    in_maps = _prep(inputs)
    res = run_bass_kernel_spmd(nc, in_maps, core_ids=list(range(8)),
                               **_CACHE.get("run_kwargs", {}))
    _CACHE["last_res"] = res
    outs = res.results

    pred = np.concatenate([outs[TP * p]["pred_out"] for p in range(DP)], axis=0)

    s = [outs[c]["partials"].astype(np.float64).sum(axis=0) for c in range(8)]
    g0 = range(TP)                       # one DP group's ranks cover all shards
    pairs_dedup = [TP * p for p in range(DP)]
    psw = sum(s[c][0] for c in g0)       # -Σls (w) + xz (zw pre-scaled by 1/TP)
    expsum = sum(s[c][1] for c in g0)    # Σ exp(ls), both w matrices
    bls = sum(s[c][2] + s[c][3] for c in g0)
    beb2 = sum(s[c][4] + s[c][5] for c in g0)
    flow = sum(s[c][6] for c in range(8))
    ce_raw = sum(s[c][8] for c in pairs_dedup)
    wsq = sum(s[c][9] for c in g0)       # -0.5Σeps² + 0.01Σmu²

    n_w = H0 * IN + H1 * H0
    n_b = H0 + H1
    L = (psw + wsq + flow
         + n_w * C0
         + n_b * C0 - bls - 0.5 * beb2
         + 60000.0 * (-ce_raw / B)
         + DECAY * expsum)
    loss = np.float32(np.float32(L) ** 2)
    return pred, loss


# revision 28
# speedup vs baseline: 92.5638x; 1.0524x over previous
"""Bass/Trainium2 kernel for nn_BaysianMLPMaskedDropout (8 NeuronCores).

Sharding: 2 data-parallel groups (batch 1024 -> 2 x 512) x 4-way tensor
parallel (the 4096 hidden axis of each BayesLinear split in quarters).
Activations are kept feature-major ("transposed", [feature, batch]) on chip so
every matmul contracts along the partition axis with no transposes between
layers.  A per-quad ReduceScatter after layer 1 and AllReduce after the output
layer run on the collective engine.  The scalar flow-loss pieces are reduced
on-device into per-core partial vectors and combined on the host.

log(pw) = log(cdf(w+eps)-cdf(w-eps)) is evaluated with the exact expansion
log(2*EPS/sqrt(2pi)) - w_ls - eps^2/2 (error < 1e-7, far below the f32 noise
of the reference's catastrophically-cancelling cdf difference).
"""

import numpy as np

import concourse.bacc as bacc
import concourse.tile as tile
from concourse import hw_specs as _hw_specs

# All ACT functions this kernel uses (Exp, Ln, Lrelu, Copy, Square) co-reside
# in the natural_log_exp_and_others table set; restrict the set chooser to it
# so the scheduler emits one table load instead of ~26 (2.7us each).
_orig_gat = _hw_specs.get_activation_tables

def _gat_one_set(arch):
    tabs = _orig_gat(arch)
    pref = "natural_log_exp_and_others"
    if pref in tabs:
        tabs = {k: (v if k == pref else set()) for k, v in tabs.items()}
    return tabs

bacc.get_activation_tables = _gat_one_set
from concourse import mybir
from concourse.alu_op_type import AluOpType
from concourse.bass_utils import run_bass_kernel_spmd

F16 = mybir.dt.float16
F32 = mybir.dt.float32
AF = mybir.ActivationFunctionType
AX = mybir.AxisListType.X

B, IN, H0, H1, OUT = 1024, 2048, 4096, 4096, 1000
OUTP = 1024                      # classes padded to 8*128 for uniform tiles
TP, DP = 4, 2
BL = B // DP                     # batch rows per DP group
NBT = BL // 128                  # batch tiles (softmax / transpose)
H0H, H1H = H0 // TP, H1 // TP    # per-rank hidden slice
NK_IN = IN // 128                # contraction chunks, layer 0
NK_H = H0H // 128                # contraction chunks, layers 1/o
G0, G1, GO = H0H // 512, H1 // 512, OUTP // 512
NBLK = G0 * NK_IN + G1 * NK_H    # weight-stream blocks with flow sums
EPS_CDF = 1e-5
C0 = float(np.log(2 * EPS_CDF / np.sqrt(2 * np.pi)))
NEG = 0.01
DECAY = 0.01
CLIP_LO = float(np.float32(1e-6))
CLIP_HI = float(np.float32(1.0 - 1e-6))

_CACHE = {}


def _build():
    nc = bacc.Bacc("TRN2", debug=False, num_devices=8, enable_asserts=False)

    def din(name, shape, dt=F16):
        return nc.dram_tensor(name, shape, dt, kind="ExternalInput").ap()

    wp0 = din("wp0", [G0 * NK_IN * 128, 1536])
    wp1 = din("wp1", [G1 * NK_H * 128, 1536])
    wpo = din("wpo", [GO * NK_H * 128, 1536])
    xT = din("xT", [IN, BL])
    pf0 = din("pf0", [H0H, BL])
    pf1 = din("pf1", [H1H, BL])
    mz0 = din("mz0", [H0H, BL])
    mz1 = din("mz1", [H1H, BL])
    NC = H0H // 128              # bias columns per layer slice
    bv0 = din("bv0", [128, 3 * NC], F32)   # [mu | ls | eps]
    bv1 = din("bv1", [128, 3 * NC], F32)
    bvo = din("bvo", [128, 24], F32)       # padded to 1024 classes
    mk0 = din("mk0", [128, NC], F32)
    mk1 = din("mk1", [128, NC], F32)
    zw2 = din("zw2", [128, NK_IN])         # f16, pre-scaled by 1/TP
    y2 = din("y2", [128, NBT], F32)
    ar1 = din("ar1", [1, OUTP], F32)
    p128 = din("p128", [128, 1], F32)

    pred_out = nc.dram_tensor("pred_out", [BL, OUT], F32, kind="ExternalOutput").ap()
    partials = nc.dram_tensor("partials", [128, 16], F32, kind="ExternalOutput").ap()

    groups = [list(range(i, i + TP)) for i in range(0, 8, TP)]

    with tile.TileContext(nc) as tc:
        with (
            tc.tile_pool(name="sb", bufs=1) as sb,
            tc.tile_pool(name="ps", bufs=1, space="PSUM") as ps,
            tc.tile_pool(name="dram", bufs=1, space="DRAM") as dram,
        ):
            # ---------------- small constant loads ----------------
            def load(name, src, shape, dt=F32):
                t = sb.tile(shape, dt, tag=name)
                nc.sync.dma_start(t[:], src)
                return t

            bv0_t = load("bv0", bv0[:, :], [128, 3 * NC])
            bv1_t = load("bv1", bv1[:, :], [128, 3 * NC])
            bvo_t = load("bvo", bvo[:, :], [128, 24])
            mk0_t = load("mk0", mk0[:, :], [128, NC])
            mk1_t = load("mk1", mk1[:, :], [128, NC])
            zw_t = load("zw", zw2[:, :], [128, NK_IN], F16)
            y_t = load("y", y2[:, :], [128, NBT])
            p128_t = load("p128", p128[:, :], [128, 1])
            ar_row = sb.tile([1, OUTP], F32, tag="ar_row")
            nc.sync.dma_start(ar_row[:], ar1[0:1, :])
            arbc = sb.tile([128, OUTP], F32, tag="arbc")
            nc.gpsimd.partition_broadcast(arbc[:], ar_row[:])
            # f16 copies of the mask columns (used as matmul lhsT for msum)
            mk0h = sb.tile([128, NC], F16, tag="mk0h")
            nc.vector.tensor_copy(mk0h[:], mk0_t[:])
            mk1h = sb.tile([128, NC], F16, tag="mk1h")
            nc.vector.tensor_copy(mk1h[:], mk1_t[:])

            negones_t = sb.tile([128, 1], F16, tag="negones")
            nc.vector.memset(negones_t[:], -1.0)

            parts_t = sb.tile([128, 16], F32, tag="parts")
            nc.vector.memset(parts_t[:], 0.0)

            # bias materialization: b = b_mu + exp(b_ls) * b_eps
            def bias_mat(bv, ncol, tag):
                s = sb.tile([128, ncol], F32, tag=tag + "s")
                nc.scalar.activation(s[:], bv[:, ncol:2 * ncol], AF.Exp)
                t = sb.tile([128, ncol], F32, tag=tag + "t")
                nc.vector.tensor_tensor(t[:], s[:], bv[:, 2 * ncol:3 * ncol], AluOpType.mult)
                b = sb.tile([128, ncol], F32, tag=tag + "b")
                nc.vector.tensor_tensor(b[:], t[:], bv[:, 0:ncol], AluOpType.add)
                return b

            b0m = bias_mat(bv0_t, NC, "b0")
            b1m = bias_mat(bv1_t, NC, "b1")
            bom = bias_mat(bvo_t, 8, "bo")

            # b-vector flow partials: sum(b_ls), sum(b_eps^2) for layers 0,1
            nc.vector.tensor_reduce(parts_t[:, 2:3], bv0_t[:, NC:2 * NC], AX, AluOpType.add)
            nc.vector.tensor_reduce(parts_t[:, 3:4], bv1_t[:, NC:2 * NC], AX, AluOpType.add)
            jb = sb.tile([128, NC], F32, tag="jb", bufs=2)
            nc.vector.affine_mul_reduce(jb[:], parts_t[:, 4:5], bv0_t[:, 2 * NC:3 * NC],
                                        bv0_t[:, 2 * NC:3 * NC], 1.0, 0.0)
            jb2 = sb.tile([128, NC], F32, tag="jb", bufs=2)
            nc.vector.affine_mul_reduce(jb2[:], parts_t[:, 5:6], bv1_t[:, 2 * NC:3 * NC],
                                        bv1_t[:, 2 * NC:3 * NC], 1.0, 0.0)

            # ---------------- x load ----------------
            xall = sb.tile([128, NK_IN * BL], F16, tag="xall")
            for k in range(NK_IN):
                nc.sync.dma_start(xall[:, k * BL:(k + 1) * BL],
                                  xT[k * 128:(k + 1) * 128, :])

            # combined scalar PSUM row: -sum(ls) + 0.01*sum(mu^2 via DVE? no –
            # ls via neg-ones matmuls) + xz/TP via zw-column matmuls
            ps_w = ps.tile([1, 512], F32, tag="psw")
            n_psw = NBLK + NK_IN   # total accumulating matmuls into ps_w
            psw_i = [0]

            def psw_mm(lhsT, rhs):
                nc.tensor.matmul(ps_w[:], lhsT, rhs, start=(psw_i[0] == 0),
                                 stop=(psw_i[0] == n_psw - 1), skip_group_check=True)
                psw_i[0] += 1

            # xz/TP partial: zw columns (pre-scaled 1/TP) against x chunks
            for k in range(NK_IN):
                psw_mm(zw_t[:, k:k + 1], xall[:, k * BL:(k + 1) * BL])

            # ---------------- masks + pfz flow (both layers) ----------------
            flacc = sb.tile([128, 4 * NK_H], F32, tag="flacc")
            ps_ms = ps.tile([1, BL], F32, tag="psms")
            ms_sb = sb.tile([1, 2 * BL], F32, tag="ms_sb")

            def mask_flow(mzap, mkh, layer):
                mzall = sb.tile([128, NK_H * BL], F16, tag=f"mzall{layer}",
                                name=f"mzall{layer}")
                for k in range(NK_H):
                    nc.sync.dma_start(mzall[:, k * BL:(k + 1) * BL],
                                      mzap[k * 128:(k + 1) * 128, :])
                    # msum partial: mask column as stationary against raw mz
                    nc.tensor.matmul(ps_ms[:], mkh[:, k:k + 1],
                                     mzall[:, k * BL:(k + 1) * BL],
                                     start=(k == 0), stop=(k == NK_H - 1))
                nc.vector.tensor_copy(ms_sb[:, layer * BL:(layer + 1) * BL], ps_ms[:])
                return mzall

            def flow_late(mzall, pfap, layer):
                pfall = sb.tile([128, NK_H * BL], F16, tag="pfall",
                                name=f"pfall{layer}")
                for k in range(NK_H):
                    nc.sync.dma_start(pfall[:, k * BL:(k + 1) * BL],
                                      pfap[k * 128:(k + 1) * 128, :])
                for k in range(NK_H):
                    mzs = mzall[:, k * BL:(k + 1) * BL]
                    pc = sb.tile([128, BL], F32, tag="pc", bufs=3)
                    nc.vector.tensor_scalar(pc[:], pfall[:, k * BL:(k + 1) * BL],
                                            CLIP_LO, CLIP_HI,
                                            AluOpType.max, AluOpType.min)
                    lnp = sb.tile([128, BL], F32, tag="lnp", bufs=3)
                    nc.scalar.activation(lnp[:], pc[:], AF.Ln)
                    ln1p = sb.tile([128, BL], F32, tag="ln1p", bufs=3)
                    nc.scalar.activation(ln1p[:], pc[:], AF.Ln, bias=1.0, scale=-1.0,
                                         accum_out=flacc[:, layer * 2 * NK_H + k:
                                                         layer * 2 * NK_H + k + 1])
                    d = sb.tile([128, BL], F32, tag="dln", bufs=3)
                    nc.vector.tensor_tensor(d[:], lnp[:], ln1p[:], AluOpType.subtract)
                    jf = sb.tile([128, BL], F32, tag="jf", bufs=2)
                    nc.vector.affine_mul_reduce(
                        jf[:], flacc[:, layer * 2 * NK_H + NK_H + k:
                                     layer * 2 * NK_H + NK_H + k + 1],
                        mzs, d[:], 1.0, 0.0)

            mz0all = mask_flow(mz0[:, :], mk0h, 0)
            mz1all = mask_flow(mz1[:, :], mk1h, 1)

            # msum AllReduce (quad) -> mult factors broadcast to 128 partitions
            cc_ms_in = dram.tile([1, 2 * BL], F32)
            cc_ms_out = dram.tile([1, 2 * BL], F32)
            nc.sync.dma_start(cc_ms_in[:], ms_sb[:])
            nc.gpsimd.collective_compute("AllReduce", AluOpType.add,
                                         replica_groups=groups,
                                         ins=[cc_ms_in.opt()], outs=[cc_ms_out.opt()])
            ms_row = sb.tile([1, 2 * BL], F32, tag="ms_row")
            nc.sync.dma_start(ms_row[:], cc_ms_out[0:1, :])
            multbc = sb.tile([128, 2 * BL], F32, tag="multbc")
            nc.gpsimd.partition_broadcast(multbc[:], ms_row[:])
            nc.vector.tensor_scalar(multbc[:], multbc[:], 1e-6, None, AluOpType.add)
            nc.vector.reciprocal(multbc[:], multbc[:])
            nc.vector.tensor_scalar(multbc[:], multbc[:], float(H0), None, AluOpType.mult)
            mult0bc = multbc[:, 0:BL]
            mult1bc = multbc[:, BL:2 * BL]

            # ---------------- streaming weight layers ----------------
            expacc = sb.tile([128, NBLK], F32, tag="expacc")
            wacc = sb.tile([128, 2 * NBLK], F32, tag="wacc")

            def wstream(wp, G, nk, rhs_sl, with_sums, blk0, evict, wtag="w",
                        wbufs=4):
                for g in range(G):
                    pts = [ps.tile([128, BL], F32, tag="psmain", bufs=5,
                                   name=f"psm{blk0}_{g}_{m}_{id(evict)}") for m in range(4)]
                    for k in range(nk):
                        blk = sb.tile([128, 1536], F16, tag="wblk", bufs=5)
                        row = (g * nk + k) * 128
                        eng = nc.sync if (k % 2 == 0) else nc.scalar
                        eng.dma_start(blk[:], wp[row:row + 128, :])
                        mu, ls, eps = blk[:, 0:512], blk[:, 512:1024], blk[:, 1024:1536]
                        s_t = sb.tile([128, 512], F16, tag="s", bufs=4)
                        bi = blk0 + g * nk + k
                        nc.scalar.activation(
                            s_t[:], ls, AF.Exp,
                            accum_out=(expacc[:, bi:bi + 1] if with_sums else None))
                        t_t = sb.tile([128, 512], F16, tag="t", bufs=4)
                        nc.vector.tensor_tensor(t_t[:], s_t[:], eps, AluOpType.mult)
                        w_t = sb.tile([128, 512], F16, tag=wtag, bufs=wbufs)
                        nc.gpsimd.tensor_tensor(w_t[:], t_t[:], mu, AluOpType.add)
                        if with_sums:
                            psw_mm(negones_t[:], ls)            # -sum(ls)
                            j1 = sb.tile([128, 512], F16, tag="j1", bufs=2)
                            nc.vector.scalar_tensor_tensor(
                                j1[:], eps, -0.5, eps, AluOpType.mult, AluOpType.mult,
                                accum_out=wacc[:, 2 * bi:2 * bi + 1])
                            j2 = sb.tile([128, 512], F16, tag="j2", bufs=2)
                            nc.vector.scalar_tensor_tensor(
                                j2[:], mu, DECAY, mu, AluOpType.mult, AluOpType.mult,
                                accum_out=wacc[:, 2 * bi + 1:2 * bi + 2])
                        for m in range(4):
                            nc.tensor.matmul(pts[m][:], w_t[:, m * 128:(m + 1) * 128],
                                             rhs_sl(k), start=(k == 0), stop=(k == nk - 1))
                    for m in range(4):
                        evict(g * 4 + m, pts[m])

            # ---- layer 0 ----
            h0m_tiles = [None] * NK_H

            def evict0(j, pt):
                lr = sb.tile([128, BL], F32, tag="h0lr", bufs=2)
                nc.scalar.activation(lr[:], pt[:], AF.Prelu, bias=b0m[:, j:j + 1],
                                     alpha=NEG)
                hm = sb.tile([128, BL], F16, tag=f"h0m{j}", name=f"h0m{j}")
                nc.vector.scalar_tensor_tensor(hm[:], lr[:], mk0_t[:, j:j + 1],
                                               mz0all[:, j * BL:(j + 1) * BL],
                                               AluOpType.mult, AluOpType.mult)
                h0m_tiles[j] = hm

            wstream(wp0, G0, NK_IN, lambda k: xall[:, k * BL:(k + 1) * BL],
                    True, 0, evict0)

            # ---- layer 1 (partials over my H0 slice, full H1) ----
            cc_h1in = dram.tile([H1, BL], F16)
            cc_h1rs = dram.tile([H1H, BL], F16)

            def evict1(j, pt):
                hp = sb.tile([128, BL], F16, tag="h1p", bufs=4)
                nc.scalar.activation(hp[:], pt[:], AF.Copy)
                nc.gpsimd.dma_start(cc_h1in[j * 128:(j + 1) * 128, :], hp[:])

            wstream(wp1, G1, NK_H, lambda k: h0m_tiles[k][:],
                    True, G0 * NK_IN, evict1)

            nc.gpsimd.collective_compute("ReduceScatter", AluOpType.add,
                                         replica_groups=groups,
                                         ins=[cc_h1in.opt()], outs=[cc_h1rs.opt()])

            h1s = sb.tile([128, NK_H * BL], F16, tag="h1s")
            for k in range(NK_H):
                nc.sync.dma_start(h1s[:, k * BL:(k + 1) * BL],
                                  cc_h1rs[k * 128:(k + 1) * 128, :])
            h1m_tiles = []
            for k in range(NK_H):
                a = sb.tile([128, BL], F32, tag="h1a", bufs=2)
                nc.vector.tensor_tensor(a[:], h1s[:, k * BL:(k + 1) * BL], mult0bc,
                                        AluOpType.mult)
                lr = sb.tile([128, BL], F32, tag="h1lr", bufs=2)
                nc.scalar.activation(lr[:], a[:], AF.Prelu, bias=b1m[:, k:k + 1],
                                     alpha=NEG)
                hm = sb.tile([128, BL], F16, tag=f"h1m{k}", name=f"h1m{k}")
                nc.vector.scalar_tensor_tensor(hm[:], lr[:], mk1_t[:, k:k + 1],
                                               mz1all[:, k * BL:(k + 1) * BL],
                                               AluOpType.mult, AluOpType.mult)
                h1m_tiles.append(hm)

            # ---- output layer ----
            cc_pin = dram.tile([OUTP, BL], F16)
            cc_pout = dram.tile([OUTP, BL], F16)

            def evicto(j, pt):
                pp = sb.tile([128, BL], F16, tag="pp", bufs=2)
                nc.scalar.activation(pp[:], pt[:], AF.Copy)
                nc.gpsimd.dma_start(cc_pin[j * 128:(j + 1) * 128, :], pp[:])
                if j == 7:
                    nc.gpsimd.collective_compute(
                        "AllReduce", AluOpType.add, replica_groups=groups,
                        ins=[cc_pin.opt()], outs=[cc_pout.opt()])

            wstream(wpo, GO, NK_H, lambda k: h1m_tiles[k][:],
                    False, 0, evicto, wtag="wo", wbufs=6)

            flow_late(mz0all, pf0[:, :], 0)
            flow_late(mz1all, pf1[:, :], 1)
            nc.vector.tensor_reduce(parts_t[:, 6:7], flacc[:, :], AX, AluOpType.add)

            par = sb.tile([128, (OUTP // 128) * BL], F16, tag="xall", name="par")
            for m in range(OUTP // 128):
                nc.sync.dma_start(par[:, m * BL:(m + 1) * BL],
                                  cc_pout[m * 128:(m + 1) * 128, :])
            predT_tiles = []
            for m in range(OUTP // 128):
                pm = sb.tile([128, BL], F32, tag="pTa", bufs=2)
                nc.vector.tensor_tensor(pm[:], par[:, m * BL:(m + 1) * BL], mult1bc,
                                        AluOpType.mult)
                pt2 = sb.tile([128, BL], F32, tag=f"pT{m}", name=f"pT{m}")
                nc.vector.tensor_scalar(pt2[:], pm[:], bom[:, m:m + 1], None,
                                        AluOpType.add)
                predT_tiles.append(pt2)

            # transpose predT [1024, BL] -> pred [BL, 1024]
            idn = sb.tile([128, 128], F32, tag="idn")
            nc.vector.tensor_scalar(idn[:], arbc[:, 0:128], p128_t[:, 0:1], None,
                                    AluOpType.is_equal)
            # softmax / CE partials per batch tile (2 rotating pred buffers)
            ceacc = sb.tile([128, NBT], F32, tag="ceacc")
            for bt in range(NBT):
                psb = sb.tile([128, OUTP], F32, tag="psb", bufs=2, name=f"psb{bt}")
                for m in range(OUTP // 128):
                    tr = ps.tile([128, 128], F32, tag="pstr", bufs=1,
                                 name=f"tr{m}_{bt}")
                    nc.tensor.transpose(tr[:], predT_tiles[m][:, bt * 128:(bt + 1) * 128],
                                        idn[:])
                    nc.scalar.activation(psb[:, m * 128:(m + 1) * 128], tr[:],
                                         AF.Copy)
                nc.sync.dma_start(pred_out[bt * 128:(bt + 1) * 128, :],
                                  psb[:, 0:OUT])
                negmax = sb.tile([128, 1], F32, tag="negmax", bufs=2)
                nc.vector.tensor_reduce(negmax[:], psb[:], AX, AluOpType.max,
                                        negate=True)
                je = sb.tile([128, OUTP], F16, tag="je", bufs=1)
                sumexp = sb.tile([128, 1], F32, tag="sumexp", bufs=2)
                nc.scalar.activation(je[:], psb[:], AF.Exp, bias=negmax[:, 0:1],
                                     accum_out=sumexp[:])
                lse = sb.tile([128, 1], F32, tag="lse", bufs=2)
                nc.scalar.activation(lse[:], sumexp[:], AF.Ln)
                iseq = sb.tile([128, OUTP], F16, tag="iseq", bufs=1)
                nc.vector.tensor_scalar(iseq[:], arbc[:], y_t[:, bt:bt + 1], None,
                                        AluOpType.is_equal)
                jp = sb.tile([128, OUTP], F32, tag="jp", bufs=1)
                py = sb.tile([128, 1], F32, tag="py", bufs=2)
                nc.vector.affine_mul_reduce(jp[:], py[:], iseq[:], psb[:],
                                            1.0, 0.0)
                tmp = sb.tile([128, 1], F32, tag="cetmp", bufs=2)
                nc.vector.tensor_tensor(tmp[:], py[:], negmax[:], AluOpType.add)
                nc.vector.tensor_tensor(ceacc[:, bt:bt + 1], tmp[:], lse[:],
                                        AluOpType.subtract)
            nc.vector.tensor_reduce(parts_t[:, 8:9], ceacc[:, :], AX, AluOpType.add)

            # final partial columns
            nc.vector.tensor_reduce(parts_t[0:1, 0:1], ps_w[:], AX, AluOpType.add)
            nc.vector.tensor_reduce(parts_t[:, 1:2], expacc[:], AX, AluOpType.add)
            nc.vector.tensor_reduce(parts_t[:, 9:10], wacc[:], AX, AluOpType.add)
            nc.sync.dma_start(partials[:, :], parts_t[:])

    nc.compile()
    return nc


def _prep(inputs):
    f16, f32 = np.float16, np.float32

    def pack_layer(mu, ls, eps, G, nk):
        # arrays [I_local, O_local] f16 -> [G*nk*128, 1536] block-contiguous
        def blocks(a):
            return a.reshape(nk, 128, G, 512).transpose(2, 0, 1, 3)
        out = np.concatenate([blocks(mu), blocks(ls), blocks(eps)], axis=3)
        return np.ascontiguousarray(out).reshape(G * nk * 128, 1536)

    packs = {}
    for r in range(TP):
        sl0 = slice(r * H0H, (r + 1) * H0H)
        packs[("wp0", r)] = pack_layer(inputs["w_mu0"].T[:, sl0].astype(f16),
                                       inputs["w_ls0"].T[:, sl0].astype(f16),
                                       inputs["eps_w0"].T[:, sl0].astype(f16),
                                       G0, NK_IN)
        packs[("wp1", r)] = pack_layer(inputs["w_mu1"].T[sl0, :].astype(f16),
                                       inputs["w_ls1"].T[sl0, :].astype(f16),
                                       inputs["eps_w1"].T[sl0, :].astype(f16),
                                       G1, NK_H)
        slo = slice(r * H1H, (r + 1) * H1H)

        def padded(a):
            out = np.zeros((H1H, OUTP), f16)
            out[:, :OUT] = a.T[slo, :].astype(f16)
            return out
        packs[("wpo", r)] = pack_layer(padded(inputs["w_muo"]),
                                       padded(inputs["w_lso"]),
                                       padded(inputs["eps_wo"]), GO, NK_H)

    def vec128(v, ncol):
        return np.ascontiguousarray(v.reshape(ncol, 128).T.astype(f32))

    NC = H0H // 128
    bvs = {}
    for r in range(TP):
        for nm in ("0", "1"):
            sl = slice(r * H0H, (r + 1) * H0H)
            bvs[(f"bv{nm}", r)] = np.concatenate(
                [vec128(inputs["b_mu" + nm][sl], NC),
                 vec128(inputs["b_ls" + nm][sl], NC),
                 vec128(inputs["eps_b" + nm][sl], NC)], axis=1)
        bmo = np.full(OUTP, -1e30, f32); bmo[:OUT] = inputs["b_muo"]
        blo = np.zeros(OUTP, f32); blo[:OUT] = inputs["b_lso"]
        beo = np.zeros(OUTP, f32); beo[:OUT] = inputs["eps_bo"]
        bvs[("bvo", r)] = np.concatenate(
            [vec128(bmo, 8), vec128(blo, 8), vec128(beo, 8)], axis=1)
        bvs[("mk0", r)] = vec128(inputs["mask_mu0"][sl].astype(f32), NC)
        bvs[("mk1", r)] = vec128(inputs["mask_mu1"][sl].astype(f32), NC)

    zw2 = np.ascontiguousarray(
        (inputs["z_w"].astype(f32) / TP).reshape(NK_IN, 128).T.astype(f16))
    ar1 = np.arange(OUTP, dtype=f32)[None, :]
    p128a = np.arange(128, dtype=f32)[:, None]

    xTf = inputs["x"].T.astype(f16)
    pf0T = inputs["pfz0"].T.astype(f16)
    pf1T = inputs["pfz1"].T.astype(f16)
    mz0T = inputs["mz0"].T.astype(f16)
    mz1T = inputs["mz1"].T.astype(f16)

    in_maps = []
    for c in range(8):
        p, r = c // TP, c % TP
        bsl = slice(p * BL, (p + 1) * BL)
        hsl = slice(r * H0H, (r + 1) * H0H)
        m = {
            "wp0": packs[("wp0", r)], "wp1": packs[("wp1", r)],
            "wpo": packs[("wpo", r)],
            "xT": np.ascontiguousarray(xTf[:, bsl]),
            "pf0": np.ascontiguousarray(pf0T[hsl, bsl]),
            "pf1": np.ascontiguousarray(pf1T[hsl, bsl]),
            "mz0": np.ascontiguousarray(mz0T[hsl, bsl]),
            "mz1": np.ascontiguousarray(mz1T[hsl, bsl]),
            "bv0": bvs[("bv0", r)], "bv1": bvs[("bv1", r)],
            "bvo": bvs[("bvo", r)],
            "mk0": bvs[("mk0", r)], "mk1": bvs[("mk1", r)],
            "zw2": zw2,
            "y2": np.ascontiguousarray(
                inputs["y"][bsl].astype(f32).reshape(NBT, 128).T),
            "ar1": ar1, "p128": p128a,
        }
        in_maps.append(m)
    return in_maps


def kernel(**inputs):
    if "nc" not in _CACHE:
        _CACHE["nc"] = _build()
    nc = _CACHE["nc"]
    in_maps = _prep(inputs)
    res = run_bass_kernel_spmd(nc, in_maps, core_ids=list(range(8)),
                               **_CACHE.get("run_kwargs", {}))
    _CACHE["last_res"] = res
    outs = res.results

    pred = np.concatenate([outs[TP * p]["pred_out"] for p in range(DP)], axis=0)

    s = [outs[c]["partials"].astype(np.float64).sum(axis=0) for c in range(8)]
    g0 = range(TP)                       # one DP group's ranks cover all shards
    pairs_dedup = [TP * p for p in range(DP)]
    psw = sum(s[c][0] for c in g0)       # -Σls (w) + xz (zw pre-scaled by 1/TP)
    expsum = sum(s[c][1] for c in g0)    # Σ exp(ls), both w matrices
    bls = sum(s[c][2] + s[c][3] for c in g0)
    beb2 = sum(s[c][4] + s[c][5] for c in g0)
    flow = sum(s[c][6] for c in range(8))
    ce_raw = sum(s[c][8] for c in pairs_dedup)
    wsq = sum(s[c][9] for c in g0)       # -0.5Σeps² + 0.01Σmu²

    n_w = H0 * IN + H1 * H0
    n_b = H0 + H1
    L = (psw + wsq + flow
         + n_w * C0
         + n_b * C0 - bls - 0.5 * beb2
         + 60000.0 * (-ce_raw / B)
         + DECAY * expsum)
    loss = np.float32(np.float32(L) ** 2)
    return pred, loss


# revision 32
# speedup vs baseline: 94.9646x; 1.0259x over previous
"""Bass/Trainium2 kernel for nn_BaysianMLPMaskedDropout (8 NeuronCores).

Sharding: 2 data-parallel groups (batch 1024 -> 2 x 512) x 4-way tensor
parallel (the 4096 hidden axis of each BayesLinear split in quarters).
Activations are kept feature-major ("transposed", [feature, batch]) on chip so
every matmul contracts along the partition axis with no transposes between
layers.  A per-quad ReduceScatter after layer 1 and AllReduce after the output
layer run on the collective engine.  The scalar flow-loss pieces are reduced
on-device into per-core partial vectors and combined on the host.

log(pw) = log(cdf(w+eps)-cdf(w-eps)) is evaluated with the exact expansion
log(2*EPS/sqrt(2pi)) - w_ls - eps^2/2 (error < 1e-7, far below the f32 noise
of the reference's catastrophically-cancelling cdf difference).
"""

import numpy as np

import concourse.bacc as bacc
import concourse.tile as tile
from concourse import hw_specs as _hw_specs

# All ACT functions this kernel uses (Exp, Ln, Lrelu, Copy, Square) co-reside
# in the natural_log_exp_and_others table set; restrict the set chooser to it
# so the scheduler emits one table load instead of ~26 (2.7us each).
_orig_gat = _hw_specs.get_activation_tables

def _gat_one_set(arch):
    tabs = _orig_gat(arch)
    pref = "natural_log_exp_and_others"
    if pref in tabs:
        tabs = {k: (v if k == pref else set()) for k, v in tabs.items()}
    return tabs

bacc.get_activation_tables = _gat_one_set
from concourse import mybir
from concourse.alu_op_type import AluOpType
from concourse.bass_utils import run_bass_kernel_spmd

F16 = mybir.dt.float16
F32 = mybir.dt.float32
AF = mybir.ActivationFunctionType
AX = mybir.AxisListType.X

B, IN, H0, H1, OUT = 1024, 2048, 4096, 4096, 1000
OUTP = 1024                      # classes padded to 8*128 for uniform tiles
TP, DP = 4, 2
BL = B // DP                     # batch rows per DP group
NBT = BL // 128                  # batch tiles (softmax / transpose)
H0H, H1H = H0 // TP, H1 // TP    # per-rank hidden slice
NK_IN = IN // 128                # contraction chunks, layer 0
NK_H = H0H // 128                # contraction chunks, layers 1/o
G0, G1, GO = H0H // 512, H1 // 512, OUTP // 512
NBLK = G0 * NK_IN + G1 * NK_H    # weight-stream blocks with flow sums
EPS_CDF = 1e-5
C0 = float(np.log(2 * EPS_CDF / np.sqrt(2 * np.pi)))
NEG = 0.01
DECAY = 0.01
CLIP_LO = float(np.float32(1e-6))
CLIP_HI = float(np.float32(1.0 - 1e-6))

_CACHE = {}


def _build():
    nc = bacc.Bacc("TRN2", debug=False, num_devices=8, enable_asserts=False)

    def din(name, shape, dt=F16):
        return nc.dram_tensor(name, shape, dt, kind="ExternalInput").ap()

    wp0 = din("wp0", [G0 * NK_IN * 128, 1536])
    wp1 = din("wp1", [G1 * NK_H * 128, 1536])
    wpo = din("wpo", [GO * NK_H * 128, 1536])
    xT = din("xT", [IN, BL])
    pf0 = din("pf0", [H0H, BL])
    pf1 = din("pf1", [H1H, BL])
    mz0 = din("mz0", [H0H, BL])
    mz1 = din("mz1", [H1H, BL])
    NC = H0H // 128              # bias columns per layer slice
    bv0 = din("bv0", [128, 3 * NC], F32)   # [mu | ls | eps]
    bv1 = din("bv1", [128, 3 * NC], F32)
    bvo = din("bvo", [128, 24], F32)       # padded to 1024 classes
    mk0 = din("mk0", [128, NC], F32)
    mk1 = din("mk1", [128, NC], F32)
    zw2 = din("zw2", [128, NK_IN])         # f16, pre-scaled by 1/TP
    y2 = din("y2", [128, NBT], F32)
    ar1 = din("ar1", [1, OUTP], F32)
    p128 = din("p128", [128, 1], F32)

    pred_out = nc.dram_tensor("pred_out", [BL, OUT], F32, kind="ExternalOutput").ap()
    partials = nc.dram_tensor("partials", [128, 16], F32, kind="ExternalOutput").ap()

    groups = [list(range(i, i + TP)) for i in range(0, 8, TP)]

    with tile.TileContext(nc) as tc:
        with (
            tc.tile_pool(name="sb", bufs=1) as sb,
            tc.tile_pool(name="ps", bufs=1, space="PSUM") as ps,
            tc.tile_pool(name="dram", bufs=1, space="DRAM") as dram,
        ):
            # ---------------- small constant loads ----------------
            def load(name, src, shape, dt=F32):
                t = sb.tile(shape, dt, tag=name)
                nc.sync.dma_start(t[:], src)
                return t

            bv0_t = load("bv0", bv0[:, :], [128, 3 * NC])
            bv1_t = load("bv1", bv1[:, :], [128, 3 * NC])
            bvo_t = load("bvo", bvo[:, :], [128, 24])
            mk0_t = load("mk0", mk0[:, :], [128, NC])
            mk1_t = load("mk1", mk1[:, :], [128, NC])
            zw_t = load("zw", zw2[:, :], [128, NK_IN], F16)
            y_t = load("y", y2[:, :], [128, NBT])
            p128_t = load("p128", p128[:, :], [128, 1])
            ar_row = sb.tile([1, OUTP], F32, tag="ar_row")
            nc.sync.dma_start(ar_row[:], ar1[0:1, :])
            arbc = sb.tile([128, OUTP], F32, tag="arbc")
            nc.gpsimd.partition_broadcast(arbc[:], ar_row[:])
            # f16 copies of the mask columns (used as matmul lhsT for msum)
            mk0h = sb.tile([128, NC], F16, tag="mk0h")
            nc.vector.tensor_copy(mk0h[:], mk0_t[:])
            mk1h = sb.tile([128, NC], F16, tag="mk1h")
            nc.vector.tensor_copy(mk1h[:], mk1_t[:])

            negones_t = sb.tile([128, 1], F16, tag="negones")
            nc.vector.memset(negones_t[:], -1.0)

            parts_t = sb.tile([128, 16], F32, tag="parts")
            nc.vector.memset(parts_t[:], 0.0)

            # bias materialization: b = b_mu + exp(b_ls) * b_eps
            def bias_mat(bv, ncol, tag):
                s = sb.tile([128, ncol], F32, tag=tag + "s")
                nc.scalar.activation(s[:], bv[:, ncol:2 * ncol], AF.Exp)
                t = sb.tile([128, ncol], F32, tag=tag + "t")
                nc.vector.tensor_tensor(t[:], s[:], bv[:, 2 * ncol:3 * ncol], AluOpType.mult)
                b = sb.tile([128, ncol], F32, tag=tag + "b")
                nc.vector.tensor_tensor(b[:], t[:], bv[:, 0:ncol], AluOpType.add)
                return b

            b0m = bias_mat(bv0_t, NC, "b0")
            b1m = bias_mat(bv1_t, NC, "b1")
            bom = bias_mat(bvo_t, 8, "bo")

            # b-vector flow partials: sum(b_ls), sum(b_eps^2) for layers 0,1
            nc.vector.tensor_reduce(parts_t[:, 2:3], bv0_t[:, NC:2 * NC], AX, AluOpType.add)
            nc.vector.tensor_reduce(parts_t[:, 3:4], bv1_t[:, NC:2 * NC], AX, AluOpType.add)
            jb = sb.tile([128, NC], F32, tag="jb", bufs=2)
            nc.vector.affine_mul_reduce(jb[:], parts_t[:, 4:5], bv0_t[:, 2 * NC:3 * NC],
                                        bv0_t[:, 2 * NC:3 * NC], 1.0, 0.0)
            jb2 = sb.tile([128, NC], F32, tag="jb", bufs=2)
            nc.vector.affine_mul_reduce(jb2[:], parts_t[:, 5:6], bv1_t[:, 2 * NC:3 * NC],
                                        bv1_t[:, 2 * NC:3 * NC], 1.0, 0.0)

            # ---------------- x load ----------------
            xall = sb.tile([128, NK_IN * BL], F16, tag="xall")
            for k in range(NK_IN):
                nc.sync.dma_start(xall[:, k * BL:(k + 1) * BL],
                                  xT[k * 128:(k + 1) * 128, :])

            # combined scalar PSUM row: -sum(ls) + 0.01*sum(mu^2 via DVE? no –
            # ls via neg-ones matmuls) + xz/TP via zw-column matmuls
            ps_w = ps.tile([1, 512], F32, tag="psw")
            n_psw = NBLK + NK_IN   # total accumulating matmuls into ps_w
            psw_i = [0]

            def psw_mm(lhsT, rhs):
                nc.tensor.matmul(ps_w[:], lhsT, rhs, start=(psw_i[0] == 0),
                                 stop=(psw_i[0] == n_psw - 1), skip_group_check=True)
                psw_i[0] += 1

            # xz/TP partial: zw columns (pre-scaled 1/TP) against x chunks
            for k in range(NK_IN):
                psw_mm(zw_t[:, k:k + 1], xall[:, k * BL:(k + 1) * BL])

            # ---------------- masks + pfz flow (both layers) ----------------
            flacc = sb.tile([128, 4 * NK_H], F32, tag="flacc")
            ps_ms = ps.tile([1, BL], F32, tag="psms")
            ms_sb = sb.tile([1, 2 * BL], F32, tag="ms_sb")

            def mask_flow(mzap, mkh, layer):
                mzall = sb.tile([128, NK_H * BL], F16, tag=f"mzall{layer}",
                                name=f"mzall{layer}")
                for k in range(NK_H):
                    nc.sync.dma_start(mzall[:, k * BL:(k + 1) * BL],
                                      mzap[k * 128:(k + 1) * 128, :])
                    # msum partial: mask column as stationary against raw mz
                    nc.tensor.matmul(ps_ms[:], mkh[:, k:k + 1],
                                     mzall[:, k * BL:(k + 1) * BL],
                                     start=(k == 0), stop=(k == NK_H - 1))
                nc.vector.tensor_copy(ms_sb[:, layer * BL:(layer + 1) * BL], ps_ms[:])
                return mzall

            def flow_late(mzall, pfap, layer):
                pfall = sb.tile([128, NK_H * BL], F16, tag="pfall",
                                name=f"pfall{layer}")
                for k in range(NK_H):
                    nc.sync.dma_start(pfall[:, k * BL:(k + 1) * BL],
                                      pfap[k * 128:(k + 1) * 128, :])
                for k in range(NK_H):
                    mzs = mzall[:, k * BL:(k + 1) * BL]
                    pc = sb.tile([128, BL], F32, tag="pc", bufs=3)
                    nc.vector.tensor_scalar(pc[:], pfall[:, k * BL:(k + 1) * BL],
                                            CLIP_LO, CLIP_HI,
                                            AluOpType.max, AluOpType.min)
                    lnp = sb.tile([128, BL], F32, tag="lnp", bufs=3)
                    nc.scalar.activation(lnp[:], pc[:], AF.Ln)
                    ln1p = sb.tile([128, BL], F32, tag="ln1p", bufs=3)
                    nc.scalar.activation(ln1p[:], pc[:], AF.Ln, bias=1.0, scale=-1.0,
                                         accum_out=flacc[:, layer * 2 * NK_H + k:
                                                         layer * 2 * NK_H + k + 1])
                    d = sb.tile([128, BL], F32, tag="dln", bufs=3)
                    nc.vector.tensor_tensor(d[:], lnp[:], ln1p[:], AluOpType.subtract)
                    jf = sb.tile([128, BL], F32, tag="jf", bufs=2)
                    nc.vector.affine_mul_reduce(
                        jf[:], flacc[:, layer * 2 * NK_H + NK_H + k:
                                     layer * 2 * NK_H + NK_H + k + 1],
                        mzs, d[:], 1.0, 0.0)

            mz0all = mask_flow(mz0[:, :], mk0h, 0)
            mz1all = mask_flow(mz1[:, :], mk1h, 1)

            # msum AllReduce (quad) -> mult factors broadcast to 128 partitions
            cc_ms_in = dram.tile([1, 2 * BL], F32)
            cc_ms_out = dram.tile([1, 2 * BL], F32)
            nc.sync.dma_start(cc_ms_in[:], ms_sb[:])
            nc.gpsimd.collective_compute("AllReduce", AluOpType.add,
                                         replica_groups=groups,
                                         ins=[cc_ms_in.opt()], outs=[cc_ms_out.opt()])
            ms_row = sb.tile([1, 2 * BL], F32, tag="ms_row")
            nc.sync.dma_start(ms_row[:], cc_ms_out[0:1, :])
            multbc = sb.tile([128, 2 * BL], F32, tag="multbc")
            nc.gpsimd.partition_broadcast(multbc[:], ms_row[:])
            nc.vector.tensor_scalar(multbc[:], multbc[:], 1e-6, None, AluOpType.add)
            nc.vector.reciprocal(multbc[:], multbc[:])
            nc.vector.tensor_scalar(multbc[:], multbc[:], float(H0), None, AluOpType.mult)
            mult0bc = multbc[:, 0:BL]
            mult1bc = multbc[:, BL:2 * BL]

            # ---------------- streaming weight layers ----------------
            expacc = sb.tile([128, NBLK], F32, tag="expacc")
            wacc = sb.tile([128, 2 * NBLK], F32, tag="wacc")

            def wstream(wp, G, nk, rhs_sl, with_sums, blk0, evict, wtag="w",
                        wbufs=4):
                for g in range(G):
                    pts = [ps.tile([128, BL], F32, tag="psmain", bufs=4,
                                   name=f"psm{blk0}_{g}_{m}_{id(evict)}") for m in range(4)]
                    for k in range(nk):
                        blk = sb.tile([128, 1536], F16, tag="wblk", bufs=5)
                        row = (g * nk + k) * 128
                        eng = nc.sync if (k % 2 == 0) else nc.scalar
                        eng.dma_start(blk[:], wp[row:row + 128, :])
                        mu, ls, eps = blk[:, 0:512], blk[:, 512:1024], blk[:, 1024:1536]
                        s_t = sb.tile([128, 512], F16, tag="s", bufs=4)
                        bi = blk0 + g * nk + k
                        nc.scalar.activation(
                            s_t[:], ls, AF.Exp,
                            accum_out=(expacc[:, bi:bi + 1] if with_sums else None))
                        t_t = sb.tile([128, 512], F16, tag="t", bufs=4)
                        nc.vector.tensor_tensor(t_t[:], s_t[:], eps, AluOpType.mult)
                        w_t = sb.tile([128, 512], F16, tag=wtag, bufs=wbufs)
                        nc.gpsimd.tensor_tensor(w_t[:], t_t[:], mu, AluOpType.add)
                        if with_sums:
                            psw_mm(negones_t[:], ls)            # -sum(ls)
                            j1 = sb.tile([128, 512], F16, tag="j1", bufs=2)
                            nc.vector.scalar_tensor_tensor(
                                j1[:], eps, -0.5, eps, AluOpType.mult, AluOpType.mult,
                                accum_out=wacc[:, 2 * bi:2 * bi + 1])
                            j2 = sb.tile([128, 512], F16, tag="j2", bufs=2)
                            nc.vector.scalar_tensor_tensor(
                                j2[:], mu, DECAY, mu, AluOpType.mult, AluOpType.mult,
                                accum_out=wacc[:, 2 * bi + 1:2 * bi + 2])
                        for m in range(4):
                            nc.tensor.matmul(pts[m][:], w_t[:, m * 128:(m + 1) * 128],
                                             rhs_sl(k), start=(k == 0), stop=(k == nk - 1))
                    for m in range(4):
                        evict(g * 4 + m, pts[m])

            # ---- layer 0 ----
            h0m_tiles = [None] * NK_H

            def evict0(j, pt):
                lr = sb.tile([128, BL], F32, tag="h0lr", bufs=2)
                nc.scalar.activation(lr[:], pt[:], AF.Prelu, bias=b0m[:, j:j + 1],
                                     alpha=NEG)
                hm = sb.tile([128, BL], F16, tag=f"h0m{j}", name=f"h0m{j}")
                nc.vector.scalar_tensor_tensor(hm[:], lr[:], mk0_t[:, j:j + 1],
                                               mz0all[:, j * BL:(j + 1) * BL],
                                               AluOpType.mult, AluOpType.mult)
                h0m_tiles[j] = hm

            wstream(wp0, G0, NK_IN, lambda k: xall[:, k * BL:(k + 1) * BL],
                    True, 0, evict0)

            # ---- layer 1 (partials over my H0 slice, full H1) ----
            cc_h1in = dram.tile([H1, BL], F16)
            cc_h1rs = dram.tile([H1H, BL], F16)

            def evict1(j, pt):
                hp = sb.tile([128, BL], F16, tag="h1p", bufs=4)
                nc.scalar.activation(hp[:], pt[:], AF.Copy)
                nc.gpsimd.dma_start(cc_h1in[j * 128:(j + 1) * 128, :], hp[:])

            wstream(wp1, G1, NK_H, lambda k: h0m_tiles[k][:],
                    True, G0 * NK_IN, evict1)

            nc.gpsimd.collective_compute("ReduceScatter", AluOpType.add,
                                         replica_groups=groups,
                                         ins=[cc_h1in.opt()], outs=[cc_h1rs.opt()])

            h1s = sb.tile([128, NK_H * BL], F16, tag="h1s")
            for k in range(NK_H):
                nc.sync.dma_start(h1s[:, k * BL:(k + 1) * BL],
                                  cc_h1rs[k * 128:(k + 1) * 128, :])
            h1m_tiles = []
            for k in range(NK_H):
                a = sb.tile([128, BL], F32, tag="h1a", bufs=2)
                nc.vector.tensor_tensor(a[:], h1s[:, k * BL:(k + 1) * BL], mult0bc,
                                        AluOpType.mult)
                lr = sb.tile([128, BL], F32, tag="h1lr", bufs=2)
                nc.scalar.activation(lr[:], a[:], AF.Prelu, bias=b1m[:, k:k + 1],
                                     alpha=NEG)
                hm = sb.tile([128, BL], F16, tag=f"h1m{k}", name=f"h1m{k}")
                nc.vector.scalar_tensor_tensor(hm[:], lr[:], mk1_t[:, k:k + 1],
                                               mz1all[:, k * BL:(k + 1) * BL],
                                               AluOpType.mult, AluOpType.mult)
                h1m_tiles.append(hm)

            # ---- output layer ----
            cc_pin = dram.tile([OUTP, BL], F16)
            cc_pout = dram.tile([OUTP, BL], F16)

            def evicto(j, pt):
                pp = sb.tile([128, BL], F16, tag="pp", bufs=2)
                nc.scalar.activation(pp[:], pt[:], AF.Copy)
                nc.gpsimd.dma_start(cc_pin[j * 128:(j + 1) * 128, :], pp[:])
                if j == 7:
                    nc.gpsimd.collective_compute(
                        "AllReduce", AluOpType.add, replica_groups=groups,
                        ins=[cc_pin.opt()], outs=[cc_pout.opt()])

            wstream(wpo, GO, NK_H, lambda k: h1m_tiles[k][:],
                    False, 0, evicto, wtag="wo", wbufs=6)

            flow_late(mz0all, pf0[:, :], 0)

            par = sb.tile([128, (OUTP // 128) * BL], F16, tag="xall", name="par")
            for m in range(OUTP // 128):
                nc.sync.dma_start(par[:, m * BL:(m + 1) * BL],
                                  cc_pout[m * 128:(m + 1) * 128, :])

            # layer-1 flow emitted here so it fills the AllReduce window
            flow_late(mz1all, pf1[:, :], 1)
            nc.vector.tensor_reduce(parts_t[:, 6:7], flacc[:, :], AX, AluOpType.add)
            predT_tiles = []
            for m in range(OUTP // 128):
                pm = sb.tile([128, BL], F32, tag="pTa", bufs=2)
                nc.vector.tensor_tensor(pm[:], par[:, m * BL:(m + 1) * BL], mult1bc,
                                        AluOpType.mult)
                pt2 = sb.tile([128, BL], F32, tag=f"pT{m}", name=f"pT{m}")
                nc.vector.tensor_scalar(pt2[:], pm[:], bom[:, m:m + 1], None,
                                        AluOpType.add)
                predT_tiles.append(pt2)

            # transpose predT [1024, BL] -> pred [BL, 1024]
            idn = sb.tile([128, 128], F32, tag="idn")
            nc.vector.tensor_scalar(idn[:], arbc[:, 0:128], p128_t[:, 0:1], None,
                                    AluOpType.is_equal)
            # softmax / CE partials per batch tile (2 rotating pred buffers)
            ceacc = sb.tile([128, NBT], F32, tag="ceacc")
            for bt in range(NBT):
                psb = sb.tile([128, OUTP], F32, tag="psb", bufs=2, name=f"psb{bt}")
                for m in range(OUTP // 128):
                    tr = ps.tile([128, 128], F32, tag="pstr", bufs=2,
                                 name=f"tr{m}_{bt}")
                    nc.tensor.transpose(tr[:], predT_tiles[m][:, bt * 128:(bt + 1) * 128],
                                        idn[:])
                    nc.scalar.activation(psb[:, m * 128:(m + 1) * 128], tr[:],
                                         AF.Copy)
                nc.sync.dma_start(pred_out[bt * 128:(bt + 1) * 128, :],
                                  psb[:, 0:OUT])
                negmax = sb.tile([128, 1], F32, tag="negmax", bufs=2)
                nc.vector.tensor_reduce(negmax[:], psb[:], AX, AluOpType.max,
                                        negate=True)
                je = sb.tile([128, OUTP], F16, tag="je", bufs=1)
                sumexp = sb.tile([128, 1], F32, tag="sumexp", bufs=2)
                nc.scalar.activation(je[:], psb[:], AF.Exp, bias=negmax[:, 0:1],
                                     accum_out=sumexp[:])
                lse = sb.tile([128, 1], F32, tag="lse", bufs=2)
                nc.scalar.activation(lse[:], sumexp[:], AF.Ln)
                iseq = sb.tile([128, OUTP], F16, tag="iseq", bufs=1)
                nc.vector.tensor_scalar(iseq[:], arbc[:], y_t[:, bt:bt + 1], None,
                                        AluOpType.is_equal)
                jp = sb.tile([128, OUTP], F32, tag="jp", bufs=1)
                py = sb.tile([128, 1], F32, tag="py", bufs=2)
                nc.vector.affine_mul_reduce(jp[:], py[:], iseq[:], psb[:],
                                            1.0, 0.0)
                tmp = sb.tile([128, 1], F32, tag="cetmp", bufs=2)
                nc.vector.tensor_tensor(tmp[:], py[:], negmax[:], AluOpType.add)
                nc.vector.tensor_tensor(ceacc[:, bt:bt + 1], tmp[:], lse[:],
                                        AluOpType.subtract)
            nc.vector.tensor_reduce(parts_t[:, 8:9], ceacc[:, :], AX, AluOpType.add)

            # final partial columns
            nc.vector.tensor_reduce(parts_t[0:1, 0:1], ps_w[:], AX, AluOpType.add)
            nc.vector.tensor_reduce(parts_t[:, 1:2], expacc[:], AX, AluOpType.add)
            nc.vector.tensor_reduce(parts_t[:, 9:10], wacc[:], AX, AluOpType.add)
            nc.sync.dma_start(partials[:, :], parts_t[:])

    nc.compile()
    return nc


def _prep(inputs):
    f16, f32 = np.float16, np.float32

    def pack_layer(mu, ls, eps, G, nk):
        # arrays [I_local, O_local] f16 -> [G*nk*128, 1536] block-contiguous
        def blocks(a):
            return a.reshape(nk, 128, G, 512).transpose(2, 0, 1, 3)
        out = np.concatenate([blocks(mu), blocks(ls), blocks(eps)], axis=3)
        return np.ascontiguousarray(out).reshape(G * nk * 128, 1536)

    packs = {}
    for r in range(TP):
        sl0 = slice(r * H0H, (r + 1) * H0H)
        packs[("wp0", r)] = pack_layer(inputs["w_mu0"].T[:, sl0].astype(f16),
                                       inputs["w_ls0"].T[:, sl0].astype(f16),
                                       inputs["eps_w0"].T[:, sl0].astype(f16),
                                       G0, NK_IN)
        packs[("wp1", r)] = pack_layer(inputs["w_mu1"].T[sl0, :].astype(f16),
                                       inputs["w_ls1"].T[sl0, :].astype(f16),
                                       inputs["eps_w1"].T[sl0, :].astype(f16),
                                       G1, NK_H)
        slo = slice(r * H1H, (r + 1) * H1H)

        def padded(a):
            out = np.zeros((H1H, OUTP), f16)
            out[:, :OUT] = a.T[slo, :].astype(f16)
            return out
        packs[("wpo", r)] = pack_layer(padded(inputs["w_muo"]),
                                       padded(inputs["w_lso"]),
                                       padded(inputs["eps_wo"]), GO, NK_H)

    def vec128(v, ncol):
        return np.ascontiguousarray(v.reshape(ncol, 128).T.astype(f32))

    NC = H0H // 128
    bvs = {}
    for r in range(TP):
        for nm in ("0", "1"):
            sl = slice(r * H0H, (r + 1) * H0H)
            bvs[(f"bv{nm}", r)] = np.concatenate(
                [vec128(inputs["b_mu" + nm][sl], NC),
                 vec128(inputs["b_ls" + nm][sl], NC),
                 vec128(inputs["eps_b" + nm][sl], NC)], axis=1)
        bmo = np.full(OUTP, -1e30, f32); bmo[:OUT] = inputs["b_muo"]
        blo = np.zeros(OUTP, f32); blo[:OUT] = inputs["b_lso"]
        beo = np.zeros(OUTP, f32); beo[:OUT] = inputs["eps_bo"]
        bvs[("bvo", r)] = np.concatenate(
            [vec128(bmo, 8), vec128(blo, 8), vec128(beo, 8)], axis=1)
        bvs[("mk0", r)] = vec128(inputs["mask_mu0"][sl].astype(f32), NC)
        bvs[("mk1", r)] = vec128(inputs["mask_mu1"][sl].astype(f32), NC)

    zw2 = np.ascontiguousarray(
        (inputs["z_w"].astype(f32) / TP).reshape(NK_IN, 128).T.astype(f16))
    ar1 = np.arange(OUTP, dtype=f32)[None, :]
    p128a = np.arange(128, dtype=f32)[:, None]

    xTf = inputs["x"].T.astype(f16)
    pf0T = inputs["pfz0"].T.astype(f16)
    pf1T = inputs["pfz1"].T.astype(f16)
    mz0T = inputs["mz0"].T.astype(f16)
    mz1T = inputs["mz1"].T.astype(f16)

    in_maps = []
    for c in range(8):
        p, r = c // TP, c % TP
        bsl = slice(p * BL, (p + 1) * BL)
        hsl = slice(r * H0H, (r + 1) * H0H)
        m = {
            "wp0": packs[("wp0", r)], "wp1": packs[("wp1", r)],
            "wpo": packs[("wpo", r)],
            "xT": np.ascontiguousarray(xTf[:, bsl]),
            "pf0": np.ascontiguousarray(pf0T[hsl, bsl]),
            "pf1": np.ascontiguousarray(pf1T[hsl, bsl]),
            "mz0": np.ascontiguousarray(mz0T[hsl, bsl]),
            "mz1": np.ascontiguousarray(mz1T[hsl, bsl]),
            "bv0": bvs[("bv0", r)], "bv1": bvs[("bv1", r)],
            "bvo": bvs[("bvo", r)],
            "mk0": bvs[("mk0", r)], "mk1": bvs[("mk1", r)],
            "zw2": zw2,
            "y2": np.ascontiguousarray(
                inputs["y"][bsl].astype(f32).reshape(NBT, 128).T),
            "ar1": ar1, "p128": p128a,
        }
        in_maps.append(m)
    return in_maps


def kernel(**inputs):
    if "nc" not in _CACHE:
        _CACHE["nc"] = _build()
    nc = _CACHE["nc"]
    in_maps = _prep(inputs)
    res = run_bass_kernel_spmd(nc, in_maps, core_ids=list(range(8)),
                               **_CACHE.get("run_kwargs", {}))
    _CACHE["last_res"] = res
    outs = res.results

    pred = np.concatenate([outs[TP * p]["pred_out"] for p in range(DP)], axis=0)

    s = [outs[c]["partials"].astype(np.float64).sum(axis=0) for c in range(8)]
    g0 = range(TP)                       # one DP group's ranks cover all shards
    pairs_dedup = [TP * p for p in range(DP)]
    psw = sum(s[c][0] for c in g0)       # -Σls (w) + xz (zw pre-scaled by 1/TP)
    expsum = sum(s[c][1] for c in g0)    # Σ exp(ls), both w matrices
    bls = sum(s[c][2] + s[c][3] for c in g0)
    beb2 = sum(s[c][4] + s[c][5] for c in g0)
    flow = sum(s[c][6] for c in range(8))
    ce_raw = sum(s[c][8] for c in pairs_dedup)
    wsq = sum(s[c][9] for c in g0)       # -0.5Σeps² + 0.01Σmu²

    n_w = H0 * IN + H1 * H0
    n_b = H0 + H1
    L = (psw + wsq + flow
         + n_w * C0
         + n_b * C0 - bls - 0.5 * beb2
         + 60000.0 * (-ce_raw / B)
         + DECAY * expsum)
    loss = np.float32(np.float32(L) ** 2)
    return pred, loss
